# revision 1
# baseline (speedup 1.0000x reference)
"""GNN message-passing (copy_u -> segment mean -> two GEMMs) on 8 trn2 NeuronCores.

Strategy (1D dst partitioning, per sharding hint):
  - Nodes are split into 392 blocks of 128; blocks are dealt to the 8 cores
    sorted by in-edge count so per-position sizes match across cores. All
    in-edges of a node live on its owner core.
  - h is shipped in bf16. Edges are gathered 128 at a time from HBM via
    dma_gather (256B rows). int16 gather indices only reach 32768 rows, so h
    is shipped as two tables (rows [0,32767) and [32767,50000)), each with a
    trailing all-zero row used for pad slots.
  - Stream packing: each core's class-k edges are packed CONTIGUOUSLY in
    (block, src) order — the 128-edge chunk grid does NOT align to block
    boundaries. A chunk on a block boundary is consumed by both blocks, each
    with its own one-hot column (foreign edges masked to PAD). This removes
    the per-(block,class) ceil padding: descriptors ~= edges, the dominant
    DMA term.
  - Aggregation per 128-edge chunk: psum[feat,dst] += G_chunk.T @ O on the
    PE (bf16), O[e,d] = (dst_local[e]==d) pure 0/1. One DVE
    tensor_tensor(is_equal) builds one-hots for OHB (block,chunk) pairs at
    once in an interleaved layout oh[e, d*OHB+c] (keeps innermost dims
    stride-1 so the DVE 2x mode applies); each matmul reads its pair as a
    stride-OHB view.
  - Mean division: hN = psum * recip[dst] during the DVE PSUM evacuation
    (tensor_tensor multiply against a [128, NPC] recip tile built once via
    GPSIMD partition_broadcast).
  - Per block: psum_out[dout,node] = W1.T @ hT_blk + W2.T @ hNT_blk (weights
    stationary, bf16; hT fully SBUF-resident so no mid-stream loads queue
    behind gather prefetch), bias added during the ScalarE PSUM evacuation.
    Output is written feature-major in bf16, two blocks per DMA (512B
    descriptors), and transposed back on the host.

Self-contained: only needs numpy + the concourse stack at /opt/trn_rl_repo.
"""

import sys

if "/opt/trn_rl_repo" not in sys.path:
    sys.path.insert(0, "/opt/trn_rl_repo")

import numpy as np
from contextlib import ExitStack

import ml_dtypes

BF16 = ml_dtypes.bfloat16

N_NODES = 50000
N_EDGES = 800000
D = 128
P = 128
NCORES = 8
NB = 49                      # blocks per core
NPC = NB * P                 # nodes per core (6272)
SPLIT = 32767                # table A holds h rows [0, SPLIT); idx SPLIT = zero row
B_ROWS = N_NODES - SPLIT     # 17233 data rows in table B
BCH = 32                     # chunks per gather batch (4096 idxs)
OHB = 8                      # (block,chunk) pairs per one-hot build
PAD_DLOC = 200.0             # dst_local sentinel -> one-hot column of zeros


def _prep(h, src, dst, W1, b1, W2, b2):
    """Host-side scheduling. Returns (in_maps, meta)."""
    src = np.asarray(src).astype(np.int64)
    dst = np.asarray(dst).astype(np.int64)
    h = np.asarray(h, dtype=np.float32)

    deg = np.bincount(dst, minlength=N_NODES).astype(np.float32)
    recip = (1.0 / np.maximum(deg, 1.0)).astype(np.float32)

    h16 = h.astype(BF16)
    tabA = np.vstack([h16[:SPLIT], np.zeros((1, D), BF16)])          # [32768, D]
    tabB = np.vstack([h16[SPLIT:], np.zeros((1, D), BF16)])          # [17234, D]

    gblk = dst // P                                   # global block id, 0..391
    slot = dst % P
    cls = (src >= SPLIT).astype(np.int64)

    # balanced assignment of blocks to (core, position): sort blocks by total
    # in-edge count, deal 8 consecutive per position so per-position sizes
    # (and hence cumulative chunk boundaries) track across cores.
    gcnt = np.bincount(gblk * 2 + cls, minlength=NCORES * NB * 2).reshape(NCORES * NB, 2)
    blk_order = np.lexsort((gcnt[:, 1], gcnt[:, 0] + gcnt[:, 1]))[::-1]
    loadsA = np.zeros(NCORES)
    loadsB = np.zeros(NCORES)
    ncnt = np.zeros(NCORES, np.int64)
    percore = [[] for _ in range(NCORES)]
    for g in blk_order:
        a, b = gcnt[g]
        cand = [c for c in range(NCORES) if ncnt[c] < NB]
        c = min(cand, key=lambda c: max(loadsA[c] + a, (loadsB[c] + b) * 2.4))
        percore[c].append(g)
        loadsA[c] += a
        loadsB[c] += b
        ncnt[c] += 1
    asg = np.empty((NB, NCORES), np.int64)
    for c in range(NCORES):
        blocks = np.array(percore[c])
        o = np.argsort(-(gcnt[blocks, 0] + gcnt[blocks, 1]))
        asg[:, c] = blocks[o]                         # asg[j, c] = global block
    # interleave big/small positions so the pipeline tail isn't a train of
    # tiny blocks
    riffle = []
    lo, hi = 0, NB - 1
    while lo <= hi:
        riffle.append(lo)
        if lo != hi:
            riffle.append(hi)
        lo += 1
        hi -= 1
    riffle = [r for r in riffle if r != NB - 1] + [NB - 1]
    asg = asg[np.array(riffle)]
    # greedy per-core realignment: order each core's blocks so its class
    # cumsums track the mean trajectory — keeps the shared chunk-grid ranges
    # (lo_r/hi_r below) tight across cores.
    cntb = gcnt[asg]                                  # [NB, NCORES, 2]
    TA = cntb[:, :, 0].mean(axis=1).cumsum()
    TB = cntb[:, :, 1].mean(axis=1).cumsum()
    asg_al = np.empty_like(asg)
    for c in range(NCORES):
        rem = list(asg[:, c])
        SA = SB = 0
        for j in range(NB):
            best = None
            bi = None
            for i, g in enumerate(rem):
                a, b = gcnt[g]
                cost = abs(SA + a - TA[j]) + abs(SB + b - TB[j])
                if best is None or cost < best:
                    best = cost
                    bi = i
            g = rem.pop(bi)
            SA += gcnt[g][0]
            SB += gcnt[g][1]
            asg_al[j, c] = g
    asg = asg_al
    pos_of = np.empty(NCORES * NB, np.int64)
    core_of = np.empty(NCORES * NB, np.int64)
    for j in range(NB):
        for c in range(NCORES):
            g = asg[j, c]
            pos_of[g] = j
            core_of[g] = c
    owner = core_of[gblk]
    blk = pos_of[gblk]

    # counts per (core, position, class); cumulative slot offsets per core
    key = ((owner * NB) + blk) * 2 + cls
    cnt = np.bincount(key, minlength=NCORES * NB * 2).reshape(NCORES, NB, 2)
    S = np.zeros((2, NCORES, NB + 1), np.int64)
    for k in (0, 1):
        S[k, :, 1:] = np.cumsum(cnt[:, :, k], axis=1)

    nch = [0, 0]
    lo_r = np.zeros((2, NB), np.int64)
    hi_r = np.zeros((2, NB), np.int64)
    for k in (0, 1):
        nch[k] = max(1, int(-(-S[k, :, -1].max() // P)))
        lo_r[k] = np.min(S[k, :, :-1], axis=0) // P
        hi_r[k] = -(-np.max(S[k, :, 1:], axis=0) // P)
        hi_r[k] = np.minimum(hi_r[k], nch[k])
        hi_r[k] = np.maximum(hi_r[k], lo_r[k])
    # (position, chunk) pair enumeration, consumption order
    pair_idx = [dict(), dict()]
    npairs = [0, 0]
    for k in (0, 1):
        n = 0
        for j in range(NB):
            for ch in range(int(lo_r[k][j]), int(hi_r[k][j])):
                pair_idx[k][(j, ch)] = n
                n += 1
        npairs[k] = n
    npairs_p = [-(-npairs[k] // OHB) * OHB for k in (0, 1)]

    # per-edge slots: edges sorted by (core, class, block, src); slot within
    # the (core, class) stream is just the running index.
    order = np.lexsort((src, blk, cls, owner))
    e_sorted = order
    ckey = owner[e_sorted] * 2 + cls[e_sorted]
    first = np.concatenate([[0], np.flatnonzero(ckey[1:] != ckey[:-1]) + 1])
    grp_change = np.concatenate([[0], np.cumsum(ckey[1:] != ckey[:-1])])
    slot_in = np.arange(len(e_sorted)) - first[grp_change]

    # interleaved iota constant: value d at column d*OHB + c
    iota_int = np.repeat(np.arange(P, dtype=np.float32), OHB)[None, :]
    iota_int = np.tile(iota_int, (P, 1)).astype(BF16)               # [128, 128*OHB]

    in_maps = []
    for c in range(NCORES):
        idxs = {
            0: np.full(nch[0] * P, SPLIT, np.int64),
            1: np.full(nch[1] * P, B_ROWS, np.int64),
        }
        dloc = {
            0: np.full((P, npairs_p[0]), PAD_DLOC, np.float32),
            1: np.full((P, npairs_p[1]), PAD_DLOC, np.float32),
        }

        mask_c = owner[e_sorted] == c
        for k in (0, 1):
            m = mask_c & (cls[e_sorted] == k)
            es = e_sorted[m]
            sl = slot_in[m]
            v = src[es] - (SPLIT if k else 0)
            idxs[k][sl] = v
            pj = blk[es]
            pch = sl // P
            pid = np.array([pair_idx[k][(int(j_), int(ch_))] for j_, ch_ in zip(pj, pch)])
            dloc[k][sl % P, pid] = slot[es]

        def idx_layout(v):
            w = v.astype(np.int16).reshape(-1, 16).T        # [16, S/16]
            return np.tile(w, (8, 1)).copy()                # [128, S/16]

        hT = np.zeros((D, NPC), np.float32)
        rcp = np.zeros((1, NPC), np.float32)
        for j in range(NB):
            g = asg[j, c]
            lo2 = g * P
            hi2 = min(lo2 + P, N_NODES)
            if hi2 > lo2:
                hT[:, j * P : j * P + (hi2 - lo2)] = h[lo2:hi2].T
                rcp[0, j * P : j * P + (hi2 - lo2)] = recip[lo2:hi2]

        auxW = np.concatenate(
            [np.asarray(W1, np.float32), np.asarray(W2, np.float32)], axis=1
        ).astype(BF16)  # [128, 256]
        auxB = (np.asarray(b1, np.float32) + np.asarray(b2, np.float32))[:, None]

        in_maps.append(
            {
                "tabA": tabA,
                "tabB": tabB,
                "idxA": idx_layout(idxs[0]),
                "idxB": idx_layout(idxs[1]),
                "dlA": dloc[0].astype(BF16),
                "dlB": dloc[1].astype(BF16),
                "hT": hT.astype(BF16),
                "rcp": rcp.astype(BF16),
                "ioI": iota_int,
                "auxW": auxW,
                "auxB": auxB.astype(np.float32),
            }
        )

    meta = dict(
        nch=nch,
        npairs=npairs,
        npairs_p=npairs_p,
        lo_r=lo_r,
        hi_r=hi_r,
        pair_idx=pair_idx,
        asg=asg,
    )
    return in_maps, meta


def _build(meta):
    import concourse.bacc as bacc
    import concourse.mybir as mybir
    import concourse.tile as tile

    nch = meta["nch"]
    npairs_p = meta["npairs_p"]
    lo_r, hi_r = meta["lo_r"], meta["hi_r"]
    pair_idx = meta["pair_idx"]
    f32 = mybir.dt.float32
    bf16 = mybir.dt.bfloat16

    nc = bacc.Bacc("TRN2", target_bir_lowering=False, debug=False, num_devices=NCORES)
    tabA = nc.declare_dram_parameter("tabA", [SPLIT + 1, D], bf16, isOutput=False)
    tabB = nc.declare_dram_parameter("tabB", [B_ROWS + 1, D], bf16, isOutput=False)
    idxA = nc.declare_dram_parameter("idxA", [P, nch[0] * 8], mybir.dt.int16, isOutput=False)
    idxB = nc.declare_dram_parameter("idxB", [P, nch[1] * 8], mybir.dt.int16, isOutput=False)
    dlA = nc.declare_dram_parameter("dlA", [P, npairs_p[0]], bf16, isOutput=False)
    dlB = nc.declare_dram_parameter("dlB", [P, npairs_p[1]], bf16, isOutput=False)
    hT = nc.declare_dram_parameter("hT", [D, NPC], bf16, isOutput=False)
    rcp = nc.declare_dram_parameter("rcp", [1, NPC], bf16, isOutput=False)
    ioI = nc.declare_dram_parameter("ioI", [P, P * OHB], bf16, isOutput=False)
    auxW = nc.declare_dram_parameter("auxW", [P, 256], bf16, isOutput=False)
    auxB = nc.declare_dram_parameter("auxB", [P, 1], f32, isOutput=False)
    outT = nc.declare_dram_parameter("outT", [D, NPC], bf16, isOutput=True)

    tabs = {0: tabA, 1: tabB}
    idx_d = {0: idxA, 1: idxB}
    dl_d = {0: dlA, 1: dlB}

    def batch_schedule(n):
        """List of (start_chunk, nchunks) gather batches: 32-wide, with the
        tail split into 8s so the pipeline drain starts sooner."""
        sched = []
        pos = 0
        while n - pos > 0:
            step = min(BCH, n - pos)
            sched.append((pos, step))
            pos += step
        # split the final batch so the tail chunks land in two waves and the
        # post-stream drain starts one wave earlier
        if sched and sched[-1][1] > 8:
            st, ln = sched.pop()
            sched.append((st, ln - 8))
            sched.append((st + ln - 8, 8))
        return sched

    bsched = {k: batch_schedule(nch[k]) for k in (0, 1)}
    bmap = {}
    for k in (0, 1):
        for bi, (st, ln) in enumerate(bsched[k]):
            for ch in range(st, st + ln):
                bmap[(k, ch)] = bi

    with tile.TileContext(nc) as tc, ExitStack() as ctx:
        consts = ctx.enter_context(tc.tile_pool(name="consts", bufs=1))
        gA_pool = ctx.enter_context(tc.tile_pool(name="gA", bufs=6))
        gB_pool = ctx.enter_context(tc.tile_pool(name="gB", bufs=4))
        ohA_pool = ctx.enter_context(tc.tile_pool(name="ohA", bufs=12))
        ohB_pool = ctx.enter_context(tc.tile_pool(name="ohB", bufs=6))
        hN_pool = ctx.enter_context(tc.tile_pool(name="hNp", bufs=4))
        out_pool = ctx.enter_context(tc.tile_pool(name="outp", bufs=8))
        psA = ctx.enter_context(tc.tile_pool(name="psA", bufs=5, space="PSUM"))
        psO = ctx.enter_context(tc.tile_pool(name="psO", bufs=3, space="PSUM"))

        # constant loads; head slices land first so the gather/onehot pipeline
        # can start while the bulk loads behind it.
        HEAD = 2 * BCH
        idx_t = {}
        dl_t = {}
        for k in (0, 1):
            it = consts.tile([P, nch[k] * 8], mybir.dt.int16, name=f"idx{k}")
            h8 = min(HEAD * 8, nch[k] * 8)
            nc.sync.dma_start(it[:, :h8], idx_d[k][:, :h8])
            if nch[k] * 8 > h8:
                nc.sync.dma_start(it[:, h8:], idx_d[k][:, h8:])
            idx_t[k] = it
            dt_ = consts.tile([P, npairs_p[k]], bf16, name=f"dl{k}")
            hd = min(HEAD, npairs_p[k])
            nc.sync.dma_start(dt_[:, :hd], dl_d[k][:, :hd])
            if npairs_p[k] > hd:
                nc.sync.dma_start(dt_[:, hd:], dl_d[k][:, hd:])
            dl_t[k] = dt_
        auxW_t = consts.tile([P, 256], bf16)
        nc.sync.dma_start(auxW_t[:], auxW[:])
        auxB_t = consts.tile([P, 1], f32)
        nc.sync.dma_start(auxB_t[:], auxB[:])
        ioI_t = consts.tile([P, P * OHB], bf16)
        nc.sync.dma_start(ioI_t[:], ioI[:])
        rcp_t = consts.tile([1, NPC], bf16)
        nc.sync.dma_start(rcp_t[:], rcp[:])
        hT_all = consts.tile([P, NPC], bf16)
        nc.sync.dma_start(hT_all[:], hT[:])
        w1_s = auxW_t[:, 0:128]
        w2_s = auxW_t[:, 128:256]
        bias_s = auxB_t[:, 0:1]

        # recip broadcast across partitions, built once on GPSIMD
        rb_t = consts.tile([P, NPC], bf16)
        nc.gpsimd.partition_broadcast(rb_t[:], rcp_t[:])

        g_pool = {0: gA_pool, 1: gB_pool}
        oh_pool = {0: ohA_pool, 1: ohB_pool}
        g_tiles = {0: {}, 1: {}}
        oh_state = {0: (-1, None), 1: (-1, None)}

        def chunk_ap(k, j):
            """AP of gathered G chunk j of stream k (emits batch gather on demand)."""
            b = bmap[(k, j)]
            st, ln = bsched[k][b]
            off = j - st
            tile_ = g_tiles[k].get(b)
            if tile_ is None:
                gt = g_pool[k].tile([P, BCH * D], bf16, name=f"g{k}")
                ni = ln * P
                nc.gpsimd.dma_gather(
                    gt[:, : ln * D].rearrange("p (c e) -> p c e", e=D),
                    tabs[k][:, :],
                    idx_t[k][:, st * 8 : st * 8 + (ni // 16)],
                    ni,
                    ni,
                    D,
                    single_packet=False,
                )
                g_tiles[k][b] = gt
                tile_ = gt
            return tile_[:, off * D : (off + 1) * D]

        def oh_ap(k, pr):
            """AP of the one-hot for pair pr of stream k ([e, d] stride-OHB view;
            emits the batched is_equal build on demand)."""
            b, ci = divmod(pr, OHB)
            cur, tile_ = oh_state[k]
            if b != cur:
                ot = oh_pool[k].tile([P, P * OHB], bf16, name=f"oh{k}")
                in1 = (
                    dl_t[k][:, b * OHB : (b + 1) * OHB]
                    .rearrange("p (x c) -> p x c", x=1)
                    .broadcast_to([P, P, OHB])
                )
                nc.vector.tensor_tensor(
                    out=ot[:].rearrange("p (d c) -> p d c", c=OHB),
                    in0=ioI_t[:].rearrange("p (d c) -> p d c", c=OHB),
                    in1=in1,
                    op=mybir.AluOpType.is_equal,
                )
                oh_state[k] = (b, ot)
                tile_ = ot
            return tile_[:].rearrange("p (d c) -> p c d", c=OHB)[:, ci, :]

        for j in range(NB):
            agg = psA.tile([P, P], f32)
            plan = [
                (k, ch, pair_idx[k][(j, ch)])
                for k in (0, 1)
                for ch in range(int(lo_r[k][j]), int(hi_r[k][j]))
            ]
            assert plan, f"empty plan at position {j}"
            for i, (k, ch, pr) in enumerate(plan):
                g_ap = chunk_ap(k, ch)
                nc.tensor.matmul(
                    agg[:],
                    lhsT=g_ap,
                    rhs=oh_ap(k, pr),
                    start=(i == 0),
                    stop=(i == len(plan) - 1),
                )

            # evacuate with the mean division folded in: hN = agg * recip[dst]
            hN_t = hN_pool.tile([P, P], bf16)
            nc.vector.tensor_tensor(
                out=hN_t[:],
                in0=agg[:],
                in1=rb_t[:, j * P : (j + 1) * P],
                op=mybir.AluOpType.mult,
            )

            po = psO.tile([P, P], f32)
            nc.tensor.matmul(
                po[:], lhsT=w1_s, rhs=hT_all[:, j * P : (j + 1) * P], start=True, stop=False
            )
            nc.tensor.matmul(po[:], lhsT=w2_s, rhs=hN_t[:], start=False, stop=True)

            if j % 2 == 0:
                out_t2 = out_pool.tile([P, 2 * P], bf16)
            out_sl = out_t2[:, 0:P] if j % 2 == 0 else out_t2[:, P : 2 * P]
            nc.scalar.activation(
                out_sl, po[:], mybir.ActivationFunctionType.Identity, bias=bias_s
            )
            if j % 2 == 1:
                nc.sync.dma_start(outT[:, (j - 1) * P : (j + 1) * P], out_t2[:])
            elif j == NB - 1:
                nc.sync.dma_start(outT[:, j * P : (j + 1) * P], out_t2[:, 0:P])

    nc.finalize()
    return nc


def kernel(h, src, dst, W1, b1, W2, b2):
    from concourse.bass_utils import run_bass_kernel_spmd

    in_maps, meta = _prep(h, src, dst, W1, b1, W2, b2)
    nc = _build(meta)
    res = run_bass_kernel_spmd(nc, in_maps, list(range(NCORES))).results
    return _assemble(res, meta)


def _assemble(res, meta):
    asg = meta["asg"]
    out = np.zeros((NCORES * NPC, D), np.float32)
    for j in range(NB):
        for c in range(NCORES):
            g = asg[j, c]
            out[g * P : (g + 1) * P] = (
                res[c]["outT"][:, j * P : (j + 1) * P].astype(np.float32).T
            )
    return out[:N_NODES].astype(np.float32)


def _sim(h, src, dst, W1, b1, W2, b2):
    """Numpy simulation of the exact device program (bookkeeping check)."""
    in_maps, meta = _prep(h, src, dst, W1, b1, W2, b2)
    nch = meta["nch"]
    lo_r, hi_r = meta["lo_r"], meta["hi_r"]
    pair_idx = meta["pair_idx"]
    outs = []
    for c in range(NCORES):
        m = in_maps[c]
        tabs = {0: m["tabA"], 1: m["tabB"]}
        idxs = {}
        dloc = {}
        for k, nm_i, nm_d in ((0, "idxA", "dlA"), (1, "idxB", "dlB")):
            w = m[nm_i][:16]
            idxs[k] = w.T.reshape(-1)
            dloc[k] = m[nm_d].astype(np.float32)
        W1b = m["auxW"][:, 0:128].astype(np.float32)
        W2b = m["auxW"][:, 128:256].astype(np.float32)
        rcpv = m["rcp"].astype(np.float32)[0]
        outT = np.zeros((D, NPC), np.float32)
        for j in range(NB):
            agg = np.zeros((P, P), np.float32)
            for k in (0, 1):
                for ch in range(int(lo_r[k][j]), int(hi_r[k][j])):
                    pr = pair_idx[k][(j, ch)]
                    gi = idxs[k][ch * P : (ch + 1) * P].astype(np.int64)
                    G = tabs[k][gi].astype(np.float32)     # [128 edges, D]
                    O = (dloc[k][:, pr, None] == np.arange(P)[None, :]).astype(
                        np.float32
                    )
                    agg += G.T @ O
            hNj = (agg * rcpv[None, j * P : (j + 1) * P]).astype(BF16).astype(np.float32)
            hTj = m["hT"][:, j * P : (j + 1) * P].astype(np.float32)
            po = W1b.T @ hTj + W2b.T @ hNj
            outT[:, j * P : (j + 1) * P] = (po + m["auxB"]).astype(BF16)
        outs.append({"outT": outT.astype(BF16)})
    return _assemble(outs, meta)


if __name__ == "__main__":
    # quick host-side validation against the reference math
    rng = np.random.default_rng(0)
    h = rng.standard_normal((N_NODES, D), dtype=np.float32)
    src = rng.integers(0, N_NODES, N_EDGES)
    dst = rng.integers(0, N_NODES, N_EDGES)
    W1 = rng.standard_normal((D, D), dtype=np.float32) * 0.1
    b1 = rng.standard_normal(D, dtype=np.float32) * 0.1
    W2 = rng.standard_normal((D, D), dtype=np.float32) * 0.1
    b2 = rng.standard_normal(D, dtype=np.float32) * 0.1

    msgs_sum = np.zeros((N_NODES, D), np.float32)
    np.add.at(msgs_sum, dst, h[src])
    deg = np.bincount(dst, minlength=N_NODES).astype(np.float32)
    hN = msgs_sum / np.maximum(deg, 1.0)[:, None]
    ref = h @ W1 + b1 + hN @ W2 + b2

    got = _sim(h, src, dst, W1, b1, W2, b2)
    err = np.linalg.norm(got - ref) / np.linalg.norm(ref)
    print("sim rel err (norm):", err)
    print("sim max abs err:", np.abs(got - ref).max())
    in_maps, meta = _prep(h, src, dst, W1, b1, W2, b2)
    print("nch:", meta["nch"], "npairs:", meta["npairs"])



# revision 16
# speedup vs baseline: 1.1243x; 1.1243x over previous
"""GNN message-passing (copy_u -> segment mean -> two GEMMs) on 8 trn2 NeuronCores.

Strategy (1D dst partitioning + paired-descriptor gather):
  - Nodes are split into 392 blocks of 128; blocks are dealt to the 8 cores
    so per-position stream sizes match across cores. All in-edges of a node
    live on its owner core.
  - The dominant cost is the per-edge dma_gather of h rows (256B descriptors,
    22.76 ns each in the cost model; <512B transfers pay a 2x latency
    multiplier, so a 512B descriptor costs the SAME 22.76 ns). Exploit: build
    HBM "pair tables" whose row q is the concatenation (h[q] | h[partner[q]])
    for a host-chosen partner map. Two edges of the same dst block whose rows
    are (q, partner[q]) share ONE 512B descriptor ("dual" slots). Unmatched
    edges gather 256B rows from the plain tables as before ("single" slots).
  - partner[] is chosen to maximize co-occurrence of (q, partner[q]) inside
    dst blocks (argmax over the block-presence Gram matrix); a greedy
    per-block capacity matcher then converts co-occurrences into matched edge
    pairs (p ~ 0.31 of edges), cutting gather descriptors by ~p/2.
  - 4 gather streams per core: single-A/B and dual-A/B (A/B split at row/pair
    32767 because dma_gather indices are int16). Each stream is packed
    contiguously in (block, key) order; the 128-slot chunk grid does not align
    to block boundaries — a chunk on a boundary is consumed by both blocks,
    each with its own one-hot column(s) (foreign slots masked to PAD).
  - Aggregation per chunk: psum[feat,dst] += G.T @ O on the PE (bf16),
    O[e,d] = (dst_local[e]==d). Dual chunks run TWO matmuls (primary half /
    partner half of the 512B slot) with two one-hot columns. One DVE
    tensor_tensor(is_equal) builds OHB member-columns at once in an
    interleaved layout oh[e, d*OHB+c].
  - Mean division folded into the DVE PSUM evacuation; per block
    psum_out[dout,node] = W1.T @ hT_blk + W2.T @ hNT_blk with weights
    stationary; bias added during the ScalarE evacuation; output written
    feature-major bf16, transposed back on host.

Self-contained: only needs numpy (+scipy if present) and /opt/trn_rl_repo.
"""

import sys

if "/opt/trn_rl_repo" not in sys.path:
    sys.path.insert(0, "/opt/trn_rl_repo")

import numpy as np
from contextlib import ExitStack

import ml_dtypes

BF16 = ml_dtypes.bfloat16

N_NODES = 50000
N_EDGES = 800000
D = 128
P = 128
NCORES = 8
NB = 49                      # blocks per core
NPC = NB * P                 # nodes per core (6272)
SPLIT = 32767                # stream-A rows/pairs are [0, SPLIT); idx SPLIT = pad
B_ROWS = N_NODES - SPLIT     # 17233 data rows/pairs in the B tables
NST = 3                      # streams: 0 single-A, 1 single-B, 2 dual (per-core table)
MC = [1, 1, 2]               # member (one-hot) columns / matmuls per chunk
EW = [D, D, 2 * D]           # gathered elements per slot
NPAIR_T = 20480              # per-core compact dual-pair table rows (pad = last)
OHB = 8                      # member columns per one-hot build
PAD_DLOC = 200.0             # dst_local sentinel -> one-hot column of zeros
BCH = [32, 32, 16]           # chunks per gather batch, per stream


def _choose_partners(src, dst):
    """partner[r] = row maximizing #dst-blocks where both rows have edges."""
    gblk = (dst // P).astype(np.int64)
    ncb = (N_NODES + P - 1) // P
    pres = np.zeros((N_NODES, ncb), np.float32)
    pres[src, gblk] = 1.0
    partner = np.full(N_NODES, -1, np.int64)
    CH = 2048
    for lo in range(0, N_NODES, CH):
        hi = min(lo + CH, N_NODES)
        W = pres[lo:hi] @ pres.T                      # [CH, N] co-occurrence
        W[np.arange(hi - lo), np.arange(lo, hi)] = -1.0
        partner[lo:hi] = np.argmax(W, axis=1)
    return partner


def _match_blocks(src, dst, partner):
    """Per dst-block greedy capacity matching.

    Returns (dual, single):
      dual:   per block b, list of (q, dst_r, dst_s) matched instances
      single: per block b, (rows, dsts) arrays of unmatched edges
    """
    gblk = (dst // P).astype(np.int64)
    ncb = NB * NCORES
    order = np.argsort(gblk * N_NODES + src, kind="stable")
    bs = gblk[order]
    rs = src[order]
    ds = dst[order]
    starts = np.searchsorted(bs, np.arange(ncb))
    ends = np.searchsorted(bs, np.arange(ncb), side="right")
    dual = [[] for _ in range(ncb)]
    single = [None] * ncb
    for b in range(ncb):
        rows = rs[starts[b]:ends[b]]
        dsts = ds[starts[b]:ends[b]]
        vals, first = np.unique(rows, return_index=True)
        cnt = {}
        ptr = {}
        for v, f in zip(vals.tolist(), first.tolist()):
            c = np.searchsorted(rows, v, side="right") - f
            cnt[v] = c
            ptr[v] = f                                # next unconsumed edge of v
        used = np.zeros(len(rows), bool)
        dl = dual[b]
        for r in vals.tolist():
            s = int(partner[r])
            if s < 0 or s == r or cnt.get(r, 0) <= 0:
                continue
            m = min(cnt[r], cnt.get(s, 0))
            if m <= 0:
                continue
            for _ in range(m):
                ir = ptr[r]
                while used[ir]:
                    ir += 1
                isx = ptr[s]
                while used[isx]:
                    isx += 1
                used[ir] = True
                used[isx] = True
                ptr[r] = ir + 1
                ptr[s] = isx + 1
                dl.append((r, int(dsts[ir]), int(dsts[isx])))
            cnt[r] -= m
            cnt[s] -= m
        keep = ~used
        single[b] = (rows[keep], dsts[keep])
    return dual, single


def _prep(h, src, dst, W1, b1, W2, b2):
    """Host-side scheduling. Returns (in_maps, meta)."""
    src = np.asarray(src).astype(np.int64)
    dst = np.asarray(dst).astype(np.int64)
    h = np.asarray(h, dtype=np.float32)

    deg = np.bincount(dst, minlength=N_NODES).astype(np.float32)
    recip = (1.0 / np.maximum(deg, 1.0)).astype(np.float32)

    partner = _choose_partners(src, dst)
    dual, single = _match_blocks(src, dst, partner)

    h16 = h.astype(BF16)
    tabA = np.vstack([h16[:SPLIT], np.zeros((1, D), BF16)])          # [32768, D]
    tabB = np.vstack([h16[SPLIT:], np.zeros((1, D), BF16)])          # [17234, D]
    # pair rows: pair q = (h[q] | h[partner[q]]); compacted per core below
    part_safe = np.where(partner >= 0, partner, 0)
    pair_all = np.concatenate([h16, h16[part_safe]], axis=1)          # [N, 256]
    pair_all[partner < 0, D:] = 0

    NGB = NB * NCORES                                # 392 grid blocks (last empty)

    # per-(global block, stream) slot lists: (key, dsts...) in sorted order
    blk_slots = [[None] * NST for _ in range(NGB)]
    w = np.zeros((NGB, NST), np.int64)
    for g in range(NGB):
        srows, sdsts = single[g]
        mA = srows < SPLIT
        blk_slots[g][0] = (srows[mA], sdsts[mA])
        blk_slots[g][1] = (srows[~mA] - SPLIT, sdsts[~mA])
        dl = dual[g]
        if dl:
            q = np.array([x[0] for x in dl], np.int64)
            d1 = np.array([x[1] % P for x in dl], np.float32)
            d2 = np.array([x[2] % P for x in dl], np.float32)
            o = np.argsort(q, kind="stable")
            blk_slots[g][2] = (q[o], d1[o], d2[o])
        else:
            z = np.zeros(0, np.int64)
            zf = np.zeros(0, np.float32)
            blk_slots[g][2] = (z, zf, zf)
        for k in range(NST):
            w[g, k] = len(blk_slots[g][k][0])

    # balanced assignment of blocks to (core, position)
    tot = w.sum(axis=0).astype(np.float64)
    tgt = np.maximum(tot / NCORES, 1.0)
    blk_order = np.argsort(-w.sum(axis=1))
    loads = np.zeros((NCORES, NST))
    ncnt = np.zeros(NCORES, np.int64)
    percore = [[] for _ in range(NCORES)]
    for g in blk_order:
        cand = [c for c in range(NCORES) if ncnt[c] < NB]
        c = min(cand, key=lambda c: np.max((loads[c] + w[g]) / tgt))
        percore[c].append(g)
        loads[c] += w[g]
        ncnt[c] += 1
    asg = np.empty((NB, NCORES), np.int64)
    for c in range(NCORES):
        blocks = np.array(percore[c])
        o = np.argsort(-w[blocks].sum(axis=1))
        asg[:, c] = blocks[o]
    # interleave big/small positions
    riffle = []
    lo, hi = 0, NB - 1
    while lo <= hi:
        riffle.append(lo)
        if lo != hi:
            riffle.append(hi)
        lo += 1
        hi -= 1
    riffle = [r for r in riffle if r != NB - 1] + [NB - 1]
    asg = asg[np.array(riffle)]
    # greedy per-core realignment so per-stream cumsums track the mean
    cntb = w[asg]                                     # [NB, NCORES, NST]
    T = cntb.mean(axis=1).cumsum(axis=0)              # [NB, NST]
    asg_al = np.empty_like(asg)
    for c in range(NCORES):
        rem = list(asg[:, c])
        S = np.zeros(NST)
        for j in range(NB):
            best = None
            bi = None
            for i, g in enumerate(rem):
                cost = np.abs(S + w[g] - T[j]).sum()
                if best is None or cost < best:
                    best = cost
                    bi = i
            g = rem.pop(bi)
            S += w[g]
            asg_al[j, c] = g
    asg = asg_al
    pos_of = np.empty(NGB, np.int64)
    core_of = np.empty(NGB, np.int64)
    for j in range(NB):
        for c in range(NCORES):
            g = asg[j, c]
            pos_of[g] = j
            core_of[g] = c

    # per-core per-stream cumulative slot offsets by position
    S = np.zeros((NST, NCORES, NB + 1), np.int64)
    for c in range(NCORES):
        for j in range(NB):
            g = asg[j, c]
            for k in range(NST):
                S[k, c, j + 1] = S[k, c, j] + w[g, k]

    nch = [0] * NST
    lo_r = np.zeros((NST, NB), np.int64)
    hi_r = np.zeros((NST, NB), np.int64)
    for k in range(NST):
        nch[k] = max(1, int(-(-S[k, :, -1].max() // P)))
        lo_r[k] = np.min(S[k, :, :-1], axis=0) // P
        hi_r[k] = -(-np.max(S[k, :, 1:], axis=0) // P)
        hi_r[k] = np.minimum(hi_r[k], nch[k])
        hi_r[k] = np.maximum(hi_r[k], lo_r[k])
    pair_idx = [dict() for _ in range(NST)]
    npairs = [0] * NST
    for k in range(NST):
        n = 0
        for j in range(NB):
            for ch in range(int(lo_r[k][j]), int(hi_r[k][j])):
                pair_idx[k][(j, ch)] = n
                n += 1
        npairs[k] = n
    # member-column counts, padded to OHB builds
    nmem = [npairs[k] * MC[k] for k in range(NST)]
    nmem_p = [-(-nmem[k] // OHB) * OHB for k in range(NST)]

    iota_int = np.repeat(np.arange(P, dtype=np.float32), OHB)[None, :]
    iota_int = np.tile(iota_int, (P, 1)).astype(BF16)               # [128, 128*OHB]

    pad_idx = [SPLIT, B_ROWS, NPAIR_T - 1]
    in_maps = []
    for c in range(NCORES):
        # compact per-core dual-pair table: global q -> local row
        qs_all = np.concatenate([blk_slots[asg[j, c]][2][0] for j in range(NB)])
        uniq = np.unique(qs_all)
        assert len(uniq) <= NPAIR_T - 1, f"dual table overflow: {len(uniq)}"
        qmap = {int(q): i for i, q in enumerate(uniq.tolist())}
        tabP = np.zeros((NPAIR_T, 2 * D), BF16)
        tabP[: len(uniq)] = pair_all[uniq]

        idxs = [np.full(nch[k] * P, pad_idx[k], np.int64) for k in range(NST)]
        dloc = [np.full((P, nmem_p[k]), PAD_DLOC, np.float32) for k in range(NST)]
        for j in range(NB):
            g = asg[j, c]
            for k in range(NST):
                sl0 = S[k, c, j]
                data = blk_slots[g][k]
                keys = data[0]
                n = len(keys)
                if n == 0:
                    continue
                sl = sl0 + np.arange(n)
                if k < 2:
                    idxs[k][sl] = keys
                else:
                    idxs[k][sl] = np.array([qmap[int(q)] for q in keys])
                pch = sl // P
                pid = np.array([pair_idx[k][(j, int(ch_))] for ch_ in pch])
                if k < 2:
                    dloc[k][sl % P, pid] = data[1] % P
                else:
                    dloc[k][sl % P, pid * 2] = data[1]
                    dloc[k][sl % P, pid * 2 + 1] = data[2]

        def idx_layout(v):
            w_ = v.astype(np.int16).reshape(-1, 16).T        # [16, S/16]
            return np.tile(w_, (8, 1)).copy()                # [128, S/16]

        hT = np.zeros((D, NPC), np.float32)
        rcp = np.zeros((1, NPC), np.float32)
        for j in range(NB):
            g = asg[j, c]
            lo2 = g * P
            hi2 = min(lo2 + P, N_NODES)
            if hi2 > lo2:
                hT[:, j * P : j * P + (hi2 - lo2)] = h[lo2:hi2].T
                rcp[0, j * P : j * P + (hi2 - lo2)] = recip[lo2:hi2]

        auxW = np.concatenate(
            [np.asarray(W1, np.float32), np.asarray(W2, np.float32)], axis=1
        ).astype(BF16)  # [128, 256]
        auxB = (np.asarray(b1, np.float32) + np.asarray(b2, np.float32))[:, None]

        in_maps.append(
            {
                "tabA": tabA,
                "tabB": tabB,
                "tabP": tabP,
                "idx0": idx_layout(idxs[0]),
                "idx1": idx_layout(idxs[1]),
                "idx2": idx_layout(idxs[2]),
                "dl0": dloc[0].astype(BF16),
                "dl1": dloc[1].astype(BF16),
                "dl2": dloc[2].astype(BF16),
                "hT": hT.astype(BF16),
                "rcp": rcp.astype(BF16),
                "ioI": iota_int,
                "auxW": auxW,
                "auxB": auxB.astype(np.float32),
            }
        )

    meta = dict(
        nch=nch,
        npairs=npairs,
        nmem=nmem,
        nmem_p=nmem_p,
        lo_r=lo_r,
        hi_r=hi_r,
        pair_idx=pair_idx,
        asg=asg,
    )
    return in_maps, meta


def _build(meta):
    import concourse.bacc as bacc
    import concourse.mybir as mybir
    import concourse.tile as tile

    nch = meta["nch"]
    nmem_p = meta["nmem_p"]
    lo_r, hi_r = meta["lo_r"], meta["hi_r"]
    pair_idx = meta["pair_idx"]
    f32 = mybir.dt.float32
    bf16 = mybir.dt.bfloat16

    nc = bacc.Bacc("TRN2", target_bir_lowering=False, debug=False, num_devices=NCORES)
    tabs_d = [
        nc.declare_dram_parameter("tabA", [SPLIT + 1, D], bf16, isOutput=False),
        nc.declare_dram_parameter("tabB", [B_ROWS + 1, D], bf16, isOutput=False),
        nc.declare_dram_parameter("tabP", [NPAIR_T, 2 * D], bf16, isOutput=False),
    ]
    idx_d = [
        nc.declare_dram_parameter(f"idx{k}", [P, nch[k] * 8], mybir.dt.int16, isOutput=False)
        for k in range(NST)
    ]
    dl_d = [
        nc.declare_dram_parameter(f"dl{k}", [P, nmem_p[k]], bf16, isOutput=False)
        for k in range(NST)
    ]
    hT = nc.declare_dram_parameter("hT", [D, NPC], bf16, isOutput=False)
    rcp = nc.declare_dram_parameter("rcp", [1, NPC], bf16, isOutput=False)
    ioI = nc.declare_dram_parameter("ioI", [P, P * OHB], bf16, isOutput=False)
    auxW = nc.declare_dram_parameter("auxW", [P, 256], bf16, isOutput=False)
    auxB = nc.declare_dram_parameter("auxB", [P, 1], f32, isOutput=False)
    outT = nc.declare_dram_parameter("outT", [D, NPC], bf16, isOutput=True)

    def batch_schedule(n, bch):
        sched = []
        pos = 0
        while n - pos > 0:
            step = min(bch, n - pos)
            sched.append((pos, step))
            pos += step
        if sched and sched[-1][1] > 8:
            st, ln = sched.pop()
            sched.append((st, ln - 8))
            sched.append((st + ln - 8, 8))
        return sched

    bsched = [batch_schedule(nch[k], BCH[k]) for k in range(NST)]
    bmap = {}
    for k in range(NST):
        for bi, (st, ln) in enumerate(bsched[k]):
            for ch in range(st, st + ln):
                bmap[(k, ch)] = bi

    with tile.TileContext(nc) as tc, ExitStack() as ctx:
        consts = ctx.enter_context(tc.tile_pool(name="consts", bufs=1))
        g_pools = [
            ctx.enter_context(tc.tile_pool(name="g0", bufs=5)),
            ctx.enter_context(tc.tile_pool(name="g1", bufs=4)),
            ctx.enter_context(tc.tile_pool(name="g2", bufs=4)),
        ]
        oh_pools = [
            ctx.enter_context(tc.tile_pool(name="oh0", bufs=8)),
            ctx.enter_context(tc.tile_pool(name="oh1", bufs=5)),
            ctx.enter_context(tc.tile_pool(name="oh2", bufs=4)),
        ]
        hN_pool = ctx.enter_context(tc.tile_pool(name="hNp", bufs=4))
        out_pool = ctx.enter_context(tc.tile_pool(name="outp", bufs=8))
        psA = ctx.enter_context(tc.tile_pool(name="psA", bufs=5, space="PSUM"))
        psO = ctx.enter_context(tc.tile_pool(name="psO", bufs=3, space="PSUM"))

        HEAD = 2 * 32
        idx_t = []
        dl_t = []
        for k in range(NST):
            it = consts.tile([P, nch[k] * 8], mybir.dt.int16, name=f"idx{k}")
            h8 = min(HEAD * 8, nch[k] * 8)
            nc.sync.dma_start(it[:, :h8], idx_d[k][:, :h8])
            if nch[k] * 8 > h8:
                nc.sync.dma_start(it[:, h8:], idx_d[k][:, h8:])
            idx_t.append(it)
            dt_ = consts.tile([P, nmem_p[k]], bf16, name=f"dl{k}")
            hd = min(HEAD, nmem_p[k])
            nc.sync.dma_start(dt_[:, :hd], dl_d[k][:, :hd])
            if nmem_p[k] > hd:
                nc.sync.dma_start(dt_[:, hd:], dl_d[k][:, hd:])
            dl_t.append(dt_)
        auxW_t = consts.tile([P, 256], bf16)
        nc.sync.dma_start(auxW_t[:], auxW[:])
        auxB_t = consts.tile([P, 1], f32)
        nc.sync.dma_start(auxB_t[:], auxB[:])
        ioI_t = consts.tile([P, P * OHB], bf16)
        nc.sync.dma_start(ioI_t[:], ioI[:])
        rcp_t = consts.tile([1, NPC], bf16)
        nc.sync.dma_start(rcp_t[:], rcp[:])
        hT_all = consts.tile([P, NPC], bf16)
        nc.sync.dma_start(hT_all[:], hT[:])
        w1_s = auxW_t[:, 0:128]
        w2_s = auxW_t[:, 128:256]
        bias_s = auxB_t[:, 0:1]

        rb_t = consts.tile([P, NPC], bf16)
        nc.gpsimd.partition_broadcast(rb_t[:], rcp_t[:])

        g_tiles = [dict() for _ in range(NST)]
        oh_state = [(-1, None) for _ in range(NST)]

        def chunk_ap(k, j):
            """AP of gathered slot-chunk j of stream k ([P, EW[k]] view)."""
            b = bmap[(k, j)]
            st, ln = bsched[k][b]
            off = j - st
            ew = EW[k]
            tile_ = g_tiles[k].get(b)
            if tile_ is None:
                gt = g_pools[k].tile([P, BCH[k] * ew], bf16, name=f"g{k}")
                ni = ln * P
                nc.gpsimd.dma_gather(
                    gt[:, : ln * ew].rearrange("p (c e) -> p c e", e=ew),
                    tabs_d[k][:, :],
                    idx_t[k][:, st * 8 : st * 8 + (ni // 16)],
                    ni,
                    ni,
                    ew,
                    single_packet=False,
                )
                g_tiles[k][b] = gt
                tile_ = gt
            return tile_[:, off * ew : (off + 1) * ew]

        def oh_ap(k, mcol):
            """AP of member column mcol of stream k (stride-OHB view)."""
            b, ci = divmod(mcol, OHB)
            cur, tile_ = oh_state[k]
            if b != cur:
                ot = oh_pools[k].tile([P, P * OHB], bf16, name=f"oh{k}")
                in1 = (
                    dl_t[k][:, b * OHB : (b + 1) * OHB]
                    .rearrange("p (x c) -> p x c", x=1)
                    .broadcast_to([P, P, OHB])
                )
                nc.vector.tensor_tensor(
                    out=ot[:].rearrange("p (d c) -> p d c", c=OHB),
                    in0=ioI_t[:].rearrange("p (d c) -> p d c", c=OHB),
                    in1=in1,
                    op=mybir.AluOpType.is_equal,
                )
                oh_state[k] = (b, ot)
                tile_ = ot
            return tile_[:].rearrange("p (d c) -> p c d", c=OHB)[:, ci, :]

        for j in range(NB):
            agg = psA.tile([P, P], f32)
            plan = []
            for k in range(NST):
                for ch in range(int(lo_r[k][j]), int(hi_r[k][j])):
                    pr = pair_idx[k][(j, ch)]
                    for i in range(MC[k]):
                        plan.append((k, ch, pr * MC[k] + i, i))
            assert plan, f"empty plan at position {j}"
            for n_, (k, ch, mcol, half) in enumerate(plan):
                g_ap = chunk_ap(k, ch)
                nc.tensor.matmul(
                    agg[:],
                    lhsT=g_ap[:, half * D : (half + 1) * D],
                    rhs=oh_ap(k, mcol),
                    start=(n_ == 0),
                    stop=(n_ == len(plan) - 1),
                )

            hN_t = hN_pool.tile([P, P], bf16)
            nc.vector.tensor_tensor(
                out=hN_t[:],
                in0=agg[:],
                in1=rb_t[:, j * P : (j + 1) * P],
                op=mybir.AluOpType.mult,
            )

            po = psO.tile([P, P], f32)
            nc.tensor.matmul(
                po[:], lhsT=w1_s, rhs=hT_all[:, j * P : (j + 1) * P], start=True, stop=False
            )
            nc.tensor.matmul(po[:], lhsT=w2_s, rhs=hN_t[:], start=False, stop=True)

            if j % 2 == 0:
                out_t2 = out_pool.tile([P, 2 * P], bf16)
            out_sl = out_t2[:, 0:P] if j % 2 == 0 else out_t2[:, P : 2 * P]
            nc.scalar.activation(
                out_sl, po[:], mybir.ActivationFunctionType.Identity, bias=bias_s
            )
            if j % 2 == 1:
                nc.sync.dma_start(outT[:, (j - 1) * P : (j + 1) * P], out_t2[:])
            elif j == NB - 1:
                nc.sync.dma_start(outT[:, j * P : (j + 1) * P], out_t2[:, 0:P])

    nc.finalize()
    return nc


def kernel(h, src, dst, W1, b1, W2, b2):
    from concourse.bass_utils import run_bass_kernel_spmd

    in_maps, meta = _prep(h, src, dst, W1, b1, W2, b2)
    nc = _build(meta)
    res = run_bass_kernel_spmd(nc, in_maps, list(range(NCORES))).results
    return _assemble(res, meta)


def _assemble(res, meta):
    asg = meta["asg"]
    out = np.zeros((NCORES * NPC, D), np.float32)
    for j in range(NB):
        for c in range(NCORES):
            g = asg[j, c]
            out[g * P : (g + 1) * P] = (
                res[c]["outT"][:, j * P : (j + 1) * P].astype(np.float32).T
            )
    return out[:N_NODES].astype(np.float32)


def _sim(h, src, dst, W1, b1, W2, b2):
    """Numpy simulation of the exact device program (bookkeeping check)."""
    in_maps, meta = _prep(h, src, dst, W1, b1, W2, b2)
    lo_r, hi_r = meta["lo_r"], meta["hi_r"]
    pair_idx = meta["pair_idx"]
    outs = []
    for c in range(NCORES):
        m = in_maps[c]
        tabs = [m["tabA"], m["tabB"], m["tabP"]]
        idxs = []
        dloc = []
        for k in range(NST):
            w_ = m[f"idx{k}"][:16]
            idxs.append(w_.T.reshape(-1))
            dloc.append(m[f"dl{k}"].astype(np.float32))
        W1b = m["auxW"][:, 0:128].astype(np.float32)
        W2b = m["auxW"][:, 128:256].astype(np.float32)
        rcpv = m["rcp"].astype(np.float32)[0]
        outT = np.zeros((D, NPC), np.float32)
        for j in range(NB):
            agg = np.zeros((P, P), np.float32)
            for k in range(NST):
                for ch in range(int(lo_r[k][j]), int(hi_r[k][j])):
                    pr = pair_idx[k][(j, ch)]
                    gi = idxs[k][ch * P : (ch + 1) * P].astype(np.int64)
                    Gfull = tabs[k][gi].astype(np.float32)   # [128, EW[k]]
                    for i in range(MC[k]):
                        G = Gfull[:, i * D : (i + 1) * D]
                        dv = dloc[k][:, (pr * MC[k] + i)]
                        O = (dv[:, None] == np.arange(P)[None, :]).astype(np.float32)
                        agg += G.T @ O
            hNj = (agg * rcpv[None, j * P : (j + 1) * P]).astype(BF16).astype(np.float32)
            hTj = m["hT"][:, j * P : (j + 1) * P].astype(np.float32)
            po = W1b.T @ hTj + W2b.T @ hNj
            outT[:, j * P : (j + 1) * P] = (po + m["auxB"]).astype(BF16)
        outs.append({"outT": outT.astype(BF16)})
    return _assemble(outs, meta)


if __name__ == "__main__":
    rng = np.random.default_rng(0)
    h = rng.standard_normal((N_NODES, D), dtype=np.float32)
    src = rng.integers(0, N_NODES, N_EDGES)
    dst = rng.integers(0, N_NODES, N_EDGES)
    W1 = rng.standard_normal((D, D), dtype=np.float32) * 0.1
    b1 = rng.standard_normal(D, dtype=np.float32) * 0.1
    W2 = rng.standard_normal((D, D), dtype=np.float32) * 0.1
    b2 = rng.standard_normal(D, dtype=np.float32) * 0.1

    msgs_sum = np.zeros((N_NODES, D), np.float32)
    np.add.at(msgs_sum, dst, h[src])
    deg = np.bincount(dst, minlength=N_NODES).astype(np.float32)
    hN = msgs_sum / np.maximum(deg, 1.0)[:, None]
    ref = h @ W1 + b1 + hN @ W2 + b2

    got = _sim(h, src, dst, W1, b1, W2, b2)
    err = np.linalg.norm(got - ref) / np.linalg.norm(ref)
    print("sim rel err (norm):", err)
    print("sim max abs err:", np.abs(got - ref).max())
    in_maps, meta = _prep(h, src, dst, W1, b1, W2, b2)
    print("nch:", meta["nch"], "npairs:", meta["npairs"])
    tot_slots = sum(meta["nch"][k] * 128 for k in range(NST))
    print("slot-descs/core:", tot_slots)


# revision 22
# speedup vs baseline: 1.2602x; 1.1209x over previous
"""GNN message-passing (copy_u -> segment mean -> two GEMMs) on 8 trn2 NeuronCores.

Strategy (1D dst partitioning + paired-descriptor gather):
  - Nodes are split into 392 blocks of 128; blocks are dealt to the 8 cores
    so per-position stream sizes match across cores. All in-edges of a node
    live on its owner core.
  - The dominant cost is the per-edge dma_gather of h rows (256B descriptors,
    22.76 ns each in the cost model; <512B transfers pay a 2x latency
    multiplier, so a 512B descriptor costs the SAME 22.76 ns). Exploit: build
    HBM "pair tables" whose row q is the concatenation (h[q] | h[partner[q]])
    for a host-chosen partner map. Two edges of the same dst block whose rows
    are (q, partner[q]) share ONE 512B descriptor ("dual" slots). Unmatched
    edges gather 256B rows from the plain tables as before ("single" slots).
  - partner[] is chosen to maximize co-occurrence of (q, partner[q]) inside
    dst blocks (argmax over the block-presence Gram matrix); a greedy
    per-block capacity matcher then converts co-occurrences into matched edge
    pairs (p ~ 0.31 of edges), cutting gather descriptors by ~p/2.
  - 4 gather streams per core: single-A/B and dual-A/B (A/B split at row/pair
    32767 because dma_gather indices are int16). Each stream is packed
    contiguously in (block, key) order; the 128-slot chunk grid does not align
    to block boundaries — a chunk on a boundary is consumed by both blocks,
    each with its own one-hot column(s) (foreign slots masked to PAD).
  - Aggregation per chunk: psum[feat,dst] += G.T @ O on the PE (bf16),
    O[e,d] = (dst_local[e]==d). Dual chunks run TWO matmuls (primary half /
    partner half of the 512B slot) with two one-hot columns. One DVE
    tensor_tensor(is_equal) builds OHB member-columns at once in an
    interleaved layout oh[e, d*OHB+c].
  - Mean division folded into the DVE PSUM evacuation; per block
    psum_out[dout,node] = W1.T @ hT_blk + W2.T @ hNT_blk with weights
    stationary; bias added during the ScalarE evacuation; output written
    feature-major bf16, transposed back on host.

Self-contained: only needs numpy (+scipy if present) and /opt/trn_rl_repo.
"""

import sys

if "/opt/trn_rl_repo" not in sys.path:
    sys.path.insert(0, "/opt/trn_rl_repo")

import numpy as np
from contextlib import ExitStack

import ml_dtypes

BF16 = ml_dtypes.bfloat16

N_NODES = 50000
N_EDGES = 800000
D = 128
P = 128
NCORES = 8
NB = 49                      # blocks per core
NPC = NB * P                 # nodes per core (6272)
SPLIT = 32767                # stream-A rows/pairs are [0, SPLIT); idx SPLIT = pad
B_ROWS = N_NODES - SPLIT     # 17233 data rows/pairs in the B tables
NST = 3                      # streams: 0 single-A, 1 single-B, 2 dual (per-core table)
MC = [1, 1, 2]               # member (one-hot) columns / matmuls per chunk
EW = [D, D, 2 * D]           # gathered elements per slot
NPAIR_T = 32768              # per-core compact dual-pair table rows (pad = last)
KPART = 3                    # partner candidates per row
OHB = 8                      # member columns per one-hot build
PAD_DLOC = 200.0             # dst_local sentinel -> one-hot column of zeros
BCH = [32, 32, 16]           # chunks per gather batch, per stream


def _choose_partners(src, dst):
    """partner[r, :K] = rows maximizing #dst-blocks where both rows have edges."""
    gblk = (dst // P).astype(np.int64)
    ncb = (N_NODES + P - 1) // P
    pres = np.zeros((N_NODES, ncb), np.float32)
    pres[src, gblk] = 1.0
    partner = np.full((N_NODES, KPART), -1, np.int64)
    CH = 2048
    for lo in range(0, N_NODES, CH):
        hi = min(lo + CH, N_NODES)
        W = pres[lo:hi] @ pres.T                      # [CH, N] co-occurrence
        W[np.arange(hi - lo), np.arange(lo, hi)] = -1.0
        top = np.argpartition(W, -KPART, axis=1)[:, -KPART:]
        vals = np.take_along_axis(W, top, axis=1)
        order = np.argsort(-vals, axis=1)
        top = np.take_along_axis(top, order, axis=1)
        vals = np.take_along_axis(vals, order, axis=1)
        top[vals < 1.0] = -1                          # useless partners
        partner[lo:hi] = top
    return partner


def _match_blocks(src, dst, partner):
    """Per dst-block greedy capacity matching over K partner candidates.

    Returns (dual, single):
      dual:   per block b, list of (q, dst_r, dst_s) matched instances,
              q = r * KPART + ki (global pair id)
      single: per block b, (rows, dsts) arrays of unmatched edges
    """
    gblk = (dst // P).astype(np.int64)
    ncb = NB * NCORES
    order = np.argsort(gblk * N_NODES + src, kind="stable")
    bs = gblk[order]
    rs = src[order]
    ds = dst[order]
    starts = np.searchsorted(bs, np.arange(ncb))
    ends = np.searchsorted(bs, np.arange(ncb), side="right")
    dual = [[] for _ in range(ncb)]
    single = [None] * ncb
    for b in range(ncb):
        rows = rs[starts[b]:ends[b]]
        dsts = ds[starts[b]:ends[b]]
        vals, first = np.unique(rows, return_index=True)
        cnt = {}
        ptr = {}
        for v, f in zip(vals.tolist(), first.tolist()):
            c = np.searchsorted(rows, v, side="right") - f
            cnt[v] = c
            ptr[v] = f                                # next unconsumed edge of v
        used = np.zeros(len(rows), bool)
        dl = dual[b]

        def take(v):
            i = ptr[v]
            while used[i]:
                i += 1
            used[i] = True
            ptr[v] = i + 1
            return i

        for r in vals.tolist():
            if cnt.get(r, 0) <= 0:
                continue
            for ki in range(KPART):
                s = int(partner[r, ki])
                if s < 0 or s == r:
                    continue
                m = min(cnt[r], cnt.get(s, 0))
                if m <= 0:
                    continue
                q = r * KPART + ki
                for _ in range(m):
                    ir = take(r)
                    isx = take(s)
                    dl.append((q, int(dsts[ir]), int(dsts[isx])))
                cnt[r] -= m
                cnt[s] -= m
                if cnt[r] <= 0:
                    break
        keep = ~used
        single[b] = (rows[keep], dsts[keep])
    return dual, single


def _prep(h, src, dst, W1, b1, W2, b2):
    """Host-side scheduling. Returns (in_maps, meta)."""
    src = np.asarray(src).astype(np.int64)
    dst = np.asarray(dst).astype(np.int64)
    h = np.asarray(h, dtype=np.float32)

    deg = np.bincount(dst, minlength=N_NODES).astype(np.float32)
    recip = (1.0 / np.maximum(deg, 1.0)).astype(np.float32)

    partner = _choose_partners(src, dst)
    dual, single = _match_blocks(src, dst, partner)

    h16 = h.astype(BF16)
    tabA = np.vstack([h16[:SPLIT], np.zeros((1, D), BF16)])          # [32768, D]
    tabB = np.vstack([h16[SPLIT:], np.zeros((1, D), BF16)])          # [17234, D]
    # pair q = r*KPART+ki -> (h[r] | h[partner[r,ki]]); compacted per core below

    NGB = NB * NCORES                                # 392 grid blocks (last empty)

    # per-(global block, stream) slot lists: (key, dsts...) in sorted order
    blk_slots = [[None] * NST for _ in range(NGB)]
    w = np.zeros((NGB, NST), np.int64)
    for g in range(NGB):
        srows, sdsts = single[g]
        mA = srows < SPLIT
        blk_slots[g][0] = (srows[mA], sdsts[mA])
        blk_slots[g][1] = (srows[~mA] - SPLIT, sdsts[~mA])
        dl = dual[g]
        if dl:
            q = np.array([x[0] for x in dl], np.int64)
            d1 = np.array([x[1] % P for x in dl], np.float32)
            d2 = np.array([x[2] % P for x in dl], np.float32)
            o = np.argsort(q, kind="stable")
            blk_slots[g][2] = (q[o], d1[o], d2[o])
        else:
            z = np.zeros(0, np.int64)
            zf = np.zeros(0, np.float32)
            blk_slots[g][2] = (z, zf, zf)
        for k in range(NST):
            w[g, k] = len(blk_slots[g][k][0])

    # balanced assignment of blocks to (core, position)
    tot = w.sum(axis=0).astype(np.float64)
    tgt = np.maximum(tot / NCORES, 1.0)
    blk_order = np.argsort(-w.sum(axis=1))
    loads = np.zeros((NCORES, NST))
    ncnt = np.zeros(NCORES, np.int64)
    percore = [[] for _ in range(NCORES)]
    for g in blk_order:
        cand = [c for c in range(NCORES) if ncnt[c] < NB]
        c = min(cand, key=lambda c: np.max((loads[c] + w[g]) / tgt))
        percore[c].append(g)
        loads[c] += w[g]
        ncnt[c] += 1
    asg = np.empty((NB, NCORES), np.int64)
    for c in range(NCORES):
        blocks = np.array(percore[c])
        o = np.argsort(-w[blocks].sum(axis=1))
        asg[:, c] = blocks[o]
    # interleave big/small positions
    riffle = []
    lo, hi = 0, NB - 1
    while lo <= hi:
        riffle.append(lo)
        if lo != hi:
            riffle.append(hi)
        lo += 1
        hi -= 1
    riffle = [r for r in riffle if r != NB - 1] + [NB - 1]
    asg = asg[np.array(riffle)]
    # greedy per-core realignment so per-stream cumsums track the mean
    cntb = w[asg]                                     # [NB, NCORES, NST]
    T = cntb.mean(axis=1).cumsum(axis=0)              # [NB, NST]
    mcw = np.array(MC, np.float64)
    asg_al = np.empty_like(asg)
    for c in range(NCORES):
        rem = list(asg[:, c])
        S = np.zeros(NST)
        for j in range(NB):
            best = None
            bi = None
            for i, g in enumerate(rem):
                cost = (np.abs(S + w[g] - T[j]) * mcw).sum()
                if best is None or cost < best:
                    best = cost
                    bi = i
            g = rem.pop(bi)
            S += w[g]
            asg_al[j, c] = g
    asg = asg_al
    pos_of = np.empty(NGB, np.int64)
    core_of = np.empty(NGB, np.int64)
    for j in range(NB):
        for c in range(NCORES):
            g = asg[j, c]
            pos_of[g] = j
            core_of[g] = c

    # per-core per-stream cumulative slot offsets by position
    S = np.zeros((NST, NCORES, NB + 1), np.int64)
    for c in range(NCORES):
        for j in range(NB):
            g = asg[j, c]
            for k in range(NST):
                S[k, c, j + 1] = S[k, c, j] + w[g, k]

    nch = [0] * NST
    lo_r = np.zeros((NST, NB), np.int64)
    hi_r = np.zeros((NST, NB), np.int64)
    for k in range(NST):
        nch[k] = max(1, int(-(-S[k, :, -1].max() // P)))
        lo_r[k] = np.min(S[k, :, :-1], axis=0) // P
        hi_r[k] = -(-np.max(S[k, :, 1:], axis=0) // P)
        hi_r[k] = np.minimum(hi_r[k], nch[k])
        hi_r[k] = np.maximum(hi_r[k], lo_r[k])
    pair_idx = [dict() for _ in range(NST)]
    npairs = [0] * NST
    for k in range(NST):
        n = 0
        for j in range(NB):
            for ch in range(int(lo_r[k][j]), int(hi_r[k][j])):
                pair_idx[k][(j, ch)] = n
                n += 1
        npairs[k] = n
    # member-column counts, padded to OHB builds
    nmem = [npairs[k] * MC[k] for k in range(NST)]
    nmem_p = [-(-nmem[k] // OHB) * OHB for k in range(NST)]

    iota_int = np.repeat(np.arange(P, dtype=np.float32), OHB)[None, :]
    iota_int = np.tile(iota_int, (P, 1)).astype(BF16)               # [128, 128*OHB]

    pad_idx = [SPLIT, B_ROWS, NPAIR_T - 1]
    in_maps = []
    for c in range(NCORES):
        # compact per-core dual-pair table: global q -> local row
        qs_all = np.concatenate([blk_slots[asg[j, c]][2][0] for j in range(NB)])
        uniq = np.unique(qs_all)
        assert len(uniq) <= NPAIR_T - 1, f"dual table overflow: {len(uniq)}"
        qmap = {int(q): i for i, q in enumerate(uniq.tolist())}
        tabP = np.zeros((NPAIR_T, 2 * D), BF16)
        r_arr = uniq // KPART
        s_arr = partner[r_arr, uniq % KPART]
        tabP[: len(uniq), :D] = h16[r_arr]
        tabP[: len(uniq), D:] = h16[s_arr]

        idxs = [np.full(nch[k] * P, pad_idx[k], np.int64) for k in range(NST)]
        dloc = [np.full((P, nmem_p[k]), PAD_DLOC, np.float32) for k in range(NST)]
        for j in range(NB):
            g = asg[j, c]
            for k in range(NST):
                sl0 = S[k, c, j]
                data = blk_slots[g][k]
                keys = data[0]
                n = len(keys)
                if n == 0:
                    continue
                sl = sl0 + np.arange(n)
                if k < 2:
                    idxs[k][sl] = keys
                else:
                    idxs[k][sl] = np.array([qmap[int(q)] for q in keys])
                pch = sl // P
                pid = np.array([pair_idx[k][(j, int(ch_))] for ch_ in pch])
                if k < 2:
                    dloc[k][sl % P, pid] = data[1] % P
                else:
                    dloc[k][sl % P, pid * 2] = data[1]
                    dloc[k][sl % P, pid * 2 + 1] = data[2]

        def idx_layout(v):
            w_ = v.astype(np.int16).reshape(-1, 16).T        # [16, S/16]
            return np.tile(w_, (8, 1)).copy()                # [128, S/16]

        hT = np.zeros((D, NPC), np.float32)
        rcp = np.zeros((1, NPC), np.float32)
        for j in range(NB):
            g = asg[j, c]
            lo2 = g * P
            hi2 = min(lo2 + P, N_NODES)
            if hi2 > lo2:
                hT[:, j * P : j * P + (hi2 - lo2)] = h[lo2:hi2].T
                rcp[0, j * P : j * P + (hi2 - lo2)] = recip[lo2:hi2]

        auxW = np.concatenate(
            [np.asarray(W1, np.float32), np.asarray(W2, np.float32)], axis=1
        ).astype(BF16)  # [128, 256]
        auxB = (np.asarray(b1, np.float32) + np.asarray(b2, np.float32))[:, None]

        in_maps.append(
            {
                "tabA": tabA,
                "tabB": tabB,
                "tabP": tabP,
                "idx0": idx_layout(idxs[0]),
                "idx1": idx_layout(idxs[1]),
                "idx2": idx_layout(idxs[2]),
                "dl0": dloc[0].astype(BF16),
                "dl1": dloc[1].astype(BF16),
                "dl2": dloc[2].astype(BF16),
                "hT": hT.astype(BF16),
                "rcp": rcp.astype(BF16),
                "ioI": iota_int,
                "auxW": auxW,
                "auxB": auxB.astype(np.float32),
            }
        )

    meta = dict(
        nch=nch,
        npairs=npairs,
        nmem=nmem,
        nmem_p=nmem_p,
        lo_r=lo_r,
        hi_r=hi_r,
        pair_idx=pair_idx,
        asg=asg,
    )
    return in_maps, meta


def _build(meta):
    import concourse.bacc as bacc
    import concourse.mybir as mybir
    import concourse.tile as tile

    nch = meta["nch"]
    nmem_p = meta["nmem_p"]
    lo_r, hi_r = meta["lo_r"], meta["hi_r"]
    pair_idx = meta["pair_idx"]
    f32 = mybir.dt.float32
    bf16 = mybir.dt.bfloat16

    nc = bacc.Bacc("TRN2", target_bir_lowering=False, debug=False, num_devices=NCORES)
    tabs_d = [
        nc.declare_dram_parameter("tabA", [SPLIT + 1, D], bf16, isOutput=False),
        nc.declare_dram_parameter("tabB", [B_ROWS + 1, D], bf16, isOutput=False),
        nc.declare_dram_parameter("tabP", [NPAIR_T, 2 * D], bf16, isOutput=False),
    ]
    idx_d = [
        nc.declare_dram_parameter(f"idx{k}", [P, nch[k] * 8], mybir.dt.int16, isOutput=False)
        for k in range(NST)
    ]
    dl_d = [
        nc.declare_dram_parameter(f"dl{k}", [P, nmem_p[k]], bf16, isOutput=False)
        for k in range(NST)
    ]
    hT = nc.declare_dram_parameter("hT", [D, NPC], bf16, isOutput=False)
    rcp = nc.declare_dram_parameter("rcp", [1, NPC], bf16, isOutput=False)
    ioI = nc.declare_dram_parameter("ioI", [P, P * OHB], bf16, isOutput=False)
    auxW = nc.declare_dram_parameter("auxW", [P, 256], bf16, isOutput=False)
    auxB = nc.declare_dram_parameter("auxB", [P, 1], f32, isOutput=False)
    outT = nc.declare_dram_parameter("outT", [D, NPC], bf16, isOutput=True)

    def batch_schedule(n, bch):
        sched = []
        pos = 0
        while n - pos > 0:
            step = min(bch, n - pos)
            sched.append((pos, step))
            pos += step
        if sched and sched[-1][1] > 8:
            st, ln = sched.pop()
            sched.append((st, ln - 8))
            sched.append((st + ln - 8, 8))
        return sched

    bsched = [batch_schedule(nch[k], BCH[k]) for k in range(NST)]
    bmap = {}
    for k in range(NST):
        for bi, (st, ln) in enumerate(bsched[k]):
            for ch in range(st, st + ln):
                bmap[(k, ch)] = bi

    with tile.TileContext(nc) as tc, ExitStack() as ctx:
        consts = ctx.enter_context(tc.tile_pool(name="consts", bufs=1))
        g_pools = [
            ctx.enter_context(tc.tile_pool(name="g0", bufs=5)),
            ctx.enter_context(tc.tile_pool(name="g1", bufs=4)),
            ctx.enter_context(tc.tile_pool(name="g2", bufs=4)),
        ]
        oh_pools = [
            ctx.enter_context(tc.tile_pool(name="oh0", bufs=8)),
            ctx.enter_context(tc.tile_pool(name="oh1", bufs=5)),
            ctx.enter_context(tc.tile_pool(name="oh2", bufs=4)),
        ]
        hN_pool = ctx.enter_context(tc.tile_pool(name="hNp", bufs=4))
        out_pool = ctx.enter_context(tc.tile_pool(name="outp", bufs=8))
        psA = ctx.enter_context(tc.tile_pool(name="psA", bufs=5, space="PSUM"))
        psO = ctx.enter_context(tc.tile_pool(name="psO", bufs=3, space="PSUM"))

        HEAD = 2 * 32
        idx_t = []
        dl_t = []
        for k in range(NST):
            it = consts.tile([P, nch[k] * 8], mybir.dt.int16, name=f"idx{k}")
            h8 = min(HEAD * 8, nch[k] * 8)
            nc.sync.dma_start(it[:, :h8], idx_d[k][:, :h8])
            if nch[k] * 8 > h8:
                nc.sync.dma_start(it[:, h8:], idx_d[k][:, h8:])
            idx_t.append(it)
            dt_ = consts.tile([P, nmem_p[k]], bf16, name=f"dl{k}")
            hd = min(HEAD, nmem_p[k])
            nc.sync.dma_start(dt_[:, :hd], dl_d[k][:, :hd])
            if nmem_p[k] > hd:
                nc.sync.dma_start(dt_[:, hd:], dl_d[k][:, hd:])
            dl_t.append(dt_)
        auxW_t = consts.tile([P, 256], bf16)
        nc.sync.dma_start(auxW_t[:], auxW[:])
        auxB_t = consts.tile([P, 1], f32)
        nc.sync.dma_start(auxB_t[:], auxB[:])
        ioI_t = consts.tile([P, P * OHB], bf16)
        nc.sync.dma_start(ioI_t[:], ioI[:])
        rcp_t = consts.tile([1, NPC], bf16)
        nc.sync.dma_start(rcp_t[:], rcp[:])
        hT_all = consts.tile([P, NPC], bf16)
        nc.sync.dma_start(hT_all[:], hT[:])
        w1_s = auxW_t[:, 0:128]
        w2_s = auxW_t[:, 128:256]
        bias_s = auxB_t[:, 0:1]

        rb_t = consts.tile([P, NPC], bf16)
        nc.gpsimd.partition_broadcast(rb_t[:], rcp_t[:])

        g_tiles = [dict() for _ in range(NST)]
        oh_state = [(-1, None) for _ in range(NST)]

        def chunk_ap(k, j):
            """AP of gathered slot-chunk j of stream k ([P, EW[k]] view)."""
            b = bmap[(k, j)]
            st, ln = bsched[k][b]
            off = j - st
            ew = EW[k]
            tile_ = g_tiles[k].get(b)
            if tile_ is None:
                gt = g_pools[k].tile([P, BCH[k] * ew], bf16, name=f"g{k}")
                ni = ln * P
                nc.gpsimd.dma_gather(
                    gt[:, : ln * ew].rearrange("p (c e) -> p c e", e=ew),
                    tabs_d[k][:, :],
                    idx_t[k][:, st * 8 : st * 8 + (ni // 16)],
                    ni,
                    ni,
                    ew,
                    single_packet=False,
                )
                g_tiles[k][b] = gt
                tile_ = gt
            return tile_[:, off * ew : (off + 1) * ew]

        def oh_ap(k, mcol):
            """AP of member column mcol of stream k (stride-OHB view)."""
            b, ci = divmod(mcol, OHB)
            cur, tile_ = oh_state[k]
            if b != cur:
                ot = oh_pools[k].tile([P, P * OHB], bf16, name=f"oh{k}")
                in1 = (
                    dl_t[k][:, b * OHB : (b + 1) * OHB]
                    .rearrange("p (x c) -> p x c", x=1)
                    .broadcast_to([P, P, OHB])
                )
                nc.vector.tensor_tensor(
                    out=ot[:].rearrange("p (d c) -> p d c", c=OHB),
                    in0=ioI_t[:].rearrange("p (d c) -> p d c", c=OHB),
                    in1=in1,
                    op=mybir.AluOpType.is_equal,
                )
                oh_state[k] = (b, ot)
                tile_ = ot
            return tile_[:].rearrange("p (d c) -> p c d", c=OHB)[:, ci, :]

        for j in range(NB):
            agg = psA.tile([P, P], f32)
            plan = []
            for k in range(NST):
                for ch in range(int(lo_r[k][j]), int(hi_r[k][j])):
                    pr = pair_idx[k][(j, ch)]
                    for i in range(MC[k]):
                        plan.append((k, ch, pr * MC[k] + i, i))
            assert plan, f"empty plan at position {j}"
            for n_, (k, ch, mcol, half) in enumerate(plan):
                g_ap = chunk_ap(k, ch)
                nc.tensor.matmul(
                    agg[:],
                    lhsT=g_ap[:, half * D : (half + 1) * D],
                    rhs=oh_ap(k, mcol),
                    start=(n_ == 0),
                    stop=(n_ == len(plan) - 1),
                )

            hN_t = hN_pool.tile([P, P], bf16)
            nc.vector.tensor_tensor(
                out=hN_t[:],
                in0=agg[:],
                in1=rb_t[:, j * P : (j + 1) * P],
                op=mybir.AluOpType.mult,
            )

            po = psO.tile([P, P], f32)
            nc.tensor.matmul(
                po[:], lhsT=w1_s, rhs=hT_all[:, j * P : (j + 1) * P], start=True, stop=False
            )
            nc.tensor.matmul(po[:], lhsT=w2_s, rhs=hN_t[:], start=False, stop=True)

            if j % 2 == 0:
                out_t2 = out_pool.tile([P, 2 * P], bf16)
            out_sl = out_t2[:, 0:P] if j % 2 == 0 else out_t2[:, P : 2 * P]
            nc.scalar.activation(
                out_sl, po[:], mybir.ActivationFunctionType.Identity, bias=bias_s
            )
            if j % 2 == 1:
                nc.sync.dma_start(outT[:, (j - 1) * P : (j + 1) * P], out_t2[:])
            elif j == NB - 1:
                nc.sync.dma_start(outT[:, j * P : (j + 1) * P], out_t2[:, 0:P])

    nc.finalize()
    return nc


def kernel(h, src, dst, W1, b1, W2, b2):
    from concourse.bass_utils import run_bass_kernel_spmd

    in_maps, meta = _prep(h, src, dst, W1, b1, W2, b2)
    nc = _build(meta)
    res = run_bass_kernel_spmd(nc, in_maps, list(range(NCORES))).results
    return _assemble(res, meta)


def _assemble(res, meta):
    asg = meta["asg"]
    out = np.zeros((NCORES * NPC, D), np.float32)
    for j in range(NB):
        for c in range(NCORES):
            g = asg[j, c]
            out[g * P : (g + 1) * P] = (
                res[c]["outT"][:, j * P : (j + 1) * P].astype(np.float32).T
            )
    return out[:N_NODES].astype(np.float32)


def _sim(h, src, dst, W1, b1, W2, b2):
    """Numpy simulation of the exact device program (bookkeeping check)."""
    in_maps, meta = _prep(h, src, dst, W1, b1, W2, b2)
    lo_r, hi_r = meta["lo_r"], meta["hi_r"]
    pair_idx = meta["pair_idx"]
    outs = []
    for c in range(NCORES):
        m = in_maps[c]
        tabs = [m["tabA"], m["tabB"], m["tabP"]]
        idxs = []
        dloc = []
        for k in range(NST):
            w_ = m[f"idx{k}"][:16]
            idxs.append(w_.T.reshape(-1))
            dloc.append(m[f"dl{k}"].astype(np.float32))
        W1b = m["auxW"][:, 0:128].astype(np.float32)
        W2b = m["auxW"][:, 128:256].astype(np.float32)
        rcpv = m["rcp"].astype(np.float32)[0]
        outT = np.zeros((D, NPC), np.float32)
        for j in range(NB):
            agg = np.zeros((P, P), np.float32)
            for k in range(NST):
                for ch in range(int(lo_r[k][j]), int(hi_r[k][j])):
                    pr = pair_idx[k][(j, ch)]
                    gi = idxs[k][ch * P : (ch + 1) * P].astype(np.int64)
                    Gfull = tabs[k][gi].astype(np.float32)   # [128, EW[k]]
                    for i in range(MC[k]):
                        G = Gfull[:, i * D : (i + 1) * D]
                        dv = dloc[k][:, (pr * MC[k] + i)]
                        O = (dv[:, None] == np.arange(P)[None, :]).astype(np.float32)
                        agg += G.T @ O
            hNj = (agg * rcpv[None, j * P : (j + 1) * P]).astype(BF16).astype(np.float32)
            hTj = m["hT"][:, j * P : (j + 1) * P].astype(np.float32)
            po = W1b.T @ hTj + W2b.T @ hNj
            outT[:, j * P : (j + 1) * P] = (po + m["auxB"]).astype(BF16)
        outs.append({"outT": outT.astype(BF16)})
    return _assemble(outs, meta)


if __name__ == "__main__":
    rng = np.random.default_rng(0)
    h = rng.standard_normal((N_NODES, D), dtype=np.float32)
    src = rng.integers(0, N_NODES, N_EDGES)
    dst = rng.integers(0, N_NODES, N_EDGES)
    W1 = rng.standard_normal((D, D), dtype=np.float32) * 0.1
    b1 = rng.standard_normal(D, dtype=np.float32) * 0.1
    W2 = rng.standard_normal((D, D), dtype=np.float32) * 0.1
    b2 = rng.standard_normal(D, dtype=np.float32) * 0.1

    msgs_sum = np.zeros((N_NODES, D), np.float32)
    np.add.at(msgs_sum, dst, h[src])
    deg = np.bincount(dst, minlength=N_NODES).astype(np.float32)
    hN = msgs_sum / np.maximum(deg, 1.0)[:, None]
    ref = h @ W1 + b1 + hN @ W2 + b2

    got = _sim(h, src, dst, W1, b1, W2, b2)
    err = np.linalg.norm(got - ref) / np.linalg.norm(ref)
    print("sim rel err (norm):", err)
    print("sim max abs err:", np.abs(got - ref).max())
    in_maps, meta = _prep(h, src, dst, W1, b1, W2, b2)
    print("nch:", meta["nch"], "npairs:", meta["npairs"])
    tot_slots = sum(meta["nch"][k] * 128 for k in range(NST))
    print("slot-descs/core:", tot_slots)


# revision 23
# speedup vs baseline: 1.2984x; 1.0303x over previous
"""GNN message-passing (copy_u -> segment mean -> two GEMMs) on 8 trn2 NeuronCores.

Strategy (1D dst partitioning + paired-descriptor gather):
  - Nodes are split into 392 blocks of 128; blocks are dealt to the 8 cores
    so per-position stream sizes match across cores. All in-edges of a node
    live on its owner core.
  - The dominant cost is the per-edge dma_gather of h rows (256B descriptors,
    22.76 ns each in the cost model; <512B transfers pay a 2x latency
    multiplier, so a 512B descriptor costs the SAME 22.76 ns). Exploit: build
    HBM "pair tables" whose row q is the concatenation (h[q] | h[partner[q]])
    for a host-chosen partner map. Two edges of the same dst block whose rows
    are (q, partner[q]) share ONE 512B descriptor ("dual" slots). Unmatched
    edges gather 256B rows from the plain tables as before ("single" slots).
  - partner[] is chosen to maximize co-occurrence of (q, partner[q]) inside
    dst blocks (argmax over the block-presence Gram matrix); a greedy
    per-block capacity matcher then converts co-occurrences into matched edge
    pairs (p ~ 0.31 of edges), cutting gather descriptors by ~p/2.
  - 4 gather streams per core: single-A/B and dual-A/B (A/B split at row/pair
    32767 because dma_gather indices are int16). Each stream is packed
    contiguously in (block, key) order; the 128-slot chunk grid does not align
    to block boundaries — a chunk on a boundary is consumed by both blocks,
    each with its own one-hot column(s) (foreign slots masked to PAD).
  - Aggregation per chunk: psum[feat,dst] += G.T @ O on the PE (bf16),
    O[e,d] = (dst_local[e]==d). Dual chunks run TWO matmuls (primary half /
    partner half of the 512B slot) with two one-hot columns. One DVE
    tensor_tensor(is_equal) builds OHB member-columns at once in an
    interleaved layout oh[e, d*OHB+c].
  - Mean division folded into the DVE PSUM evacuation; per block
    psum_out[dout,node] = W1.T @ hT_blk + W2.T @ hNT_blk with weights
    stationary; bias added during the ScalarE evacuation; output written
    feature-major bf16, transposed back on host.

Self-contained: only needs numpy (+scipy if present) and /opt/trn_rl_repo.
"""

import sys

if "/opt/trn_rl_repo" not in sys.path:
    sys.path.insert(0, "/opt/trn_rl_repo")

import numpy as np
from contextlib import ExitStack

import ml_dtypes

BF16 = ml_dtypes.bfloat16

N_NODES = 50000
N_EDGES = 800000
D = 128
P = 128
NCORES = 8
NB = 49                      # blocks per core
NPC = NB * P                 # nodes per core (6272)
SPLIT = 32767                # stream-A rows/pairs are [0, SPLIT); idx SPLIT = pad
B_ROWS = N_NODES - SPLIT     # 17233 data rows/pairs in the B tables
NST = 3                      # streams: 0 single-A, 1 single-B, 2 dual (per-core table)
MC = [1, 1, 2]               # member (one-hot) columns / matmuls per chunk
EW = [D, D, 2 * D]           # gathered elements per slot
NPAIR_T = 32768              # per-core compact dual-pair table rows (pad = last)
KPART = 5                    # partner candidates per row
OHB = 8                      # member columns per one-hot build
PAD_DLOC = 200.0             # dst_local sentinel -> one-hot column of zeros
BCH = [32, 32, 16]           # chunks per gather batch, per stream


def _choose_partners(src, dst):
    """partner[r, :K] = rows maximizing #dst-blocks where both rows have edges."""
    gblk = (dst // P).astype(np.int64)
    ncb = (N_NODES + P - 1) // P
    pres = np.zeros((N_NODES, ncb), np.float32)
    pres[src, gblk] = 1.0
    partner = np.full((N_NODES, KPART), -1, np.int64)
    CH = 2048
    for lo in range(0, N_NODES, CH):
        hi = min(lo + CH, N_NODES)
        W = pres[lo:hi] @ pres.T                      # [CH, N] co-occurrence
        W[np.arange(hi - lo), np.arange(lo, hi)] = -1.0
        top = np.argpartition(W, -KPART, axis=1)[:, -KPART:]
        vals = np.take_along_axis(W, top, axis=1)
        order = np.argsort(-vals, axis=1)
        top = np.take_along_axis(top, order, axis=1)
        vals = np.take_along_axis(vals, order, axis=1)
        top[vals < 1.0] = -1                          # useless partners
        partner[lo:hi] = top
    return partner


def _match_blocks(src, dst, partner):
    """Per dst-block greedy capacity matching over K partner candidates.

    Returns (dual, single):
      dual:   per block b, list of (q, dst_r, dst_s) matched instances,
              q = r * KPART + ki (global pair id)
      single: per block b, (rows, dsts) arrays of unmatched edges
    """
    gblk = (dst // P).astype(np.int64)
    ncb = NB * NCORES
    order = np.argsort(gblk * N_NODES + src, kind="stable")
    bs = gblk[order]
    rs = src[order]
    ds = dst[order]
    starts = np.searchsorted(bs, np.arange(ncb))
    ends = np.searchsorted(bs, np.arange(ncb), side="right")
    dual = [[] for _ in range(ncb)]
    single = [None] * ncb
    for b in range(ncb):
        rows = rs[starts[b]:ends[b]]
        dsts = ds[starts[b]:ends[b]]
        vals, first = np.unique(rows, return_index=True)
        cnt = {}
        ptr = {}
        for v, f in zip(vals.tolist(), first.tolist()):
            c = np.searchsorted(rows, v, side="right") - f
            cnt[v] = c
            ptr[v] = f                                # next unconsumed edge of v
        used = np.zeros(len(rows), bool)
        dl = dual[b]

        def take(v):
            i = ptr[v]
            while used[i]:
                i += 1
            used[i] = True
            ptr[v] = i + 1
            return i

        for r in vals.tolist():
            if cnt.get(r, 0) <= 0:
                continue
            for ki in range(KPART):
                s = int(partner[r, ki])
                if s < 0 or s == r:
                    continue
                m = min(cnt[r], cnt.get(s, 0))
                if m <= 0:
                    continue
                q = r * KPART + ki
                for _ in range(m):
                    ir = take(r)
                    isx = take(s)
                    dl.append((q, int(dsts[ir]), int(dsts[isx])))
                cnt[r] -= m
                cnt[s] -= m
                if cnt[r] <= 0:
                    break
        keep = ~used
        single[b] = (rows[keep], dsts[keep])
    return dual, single


def _prep(h, src, dst, W1, b1, W2, b2):
    """Host-side scheduling. Returns (in_maps, meta)."""
    src = np.asarray(src).astype(np.int64)
    dst = np.asarray(dst).astype(np.int64)
    h = np.asarray(h, dtype=np.float32)

    deg = np.bincount(dst, minlength=N_NODES).astype(np.float32)
    recip = (1.0 / np.maximum(deg, 1.0)).astype(np.float32)

    partner = _choose_partners(src, dst)
    dual, single = _match_blocks(src, dst, partner)

    h16 = h.astype(BF16)
    tabA = np.vstack([h16[:SPLIT], np.zeros((1, D), BF16)])          # [32768, D]
    tabB = np.vstack([h16[SPLIT:], np.zeros((1, D), BF16)])          # [17234, D]
    # pair q = r*KPART+ki -> (h[r] | h[partner[r,ki]]); compacted per core below

    NGB = NB * NCORES                                # 392 grid blocks (last empty)

    # per-(global block, stream) slot lists: (key, dsts...) in sorted order
    blk_slots = [[None] * NST for _ in range(NGB)]
    w = np.zeros((NGB, NST), np.int64)
    for g in range(NGB):
        srows, sdsts = single[g]
        mA = srows < SPLIT
        blk_slots[g][0] = (srows[mA], sdsts[mA])
        blk_slots[g][1] = (srows[~mA] - SPLIT, sdsts[~mA])
        dl = dual[g]
        if dl:
            q = np.array([x[0] for x in dl], np.int64)
            d1 = np.array([x[1] % P for x in dl], np.float32)
            d2 = np.array([x[2] % P for x in dl], np.float32)
            o = np.argsort(q, kind="stable")
            blk_slots[g][2] = (q[o], d1[o], d2[o])
        else:
            z = np.zeros(0, np.int64)
            zf = np.zeros(0, np.float32)
            blk_slots[g][2] = (z, zf, zf)
        for k in range(NST):
            w[g, k] = len(blk_slots[g][k][0])

    # balanced assignment of blocks to (core, position)
    tot = w.sum(axis=0).astype(np.float64)
    tgt = np.maximum(tot / NCORES, 1.0)
    blk_order = np.argsort(-w.sum(axis=1))
    loads = np.zeros((NCORES, NST))
    ncnt = np.zeros(NCORES, np.int64)
    percore = [[] for _ in range(NCORES)]
    for g in blk_order:
        cand = [c for c in range(NCORES) if ncnt[c] < NB]
        c = min(cand, key=lambda c: np.max((loads[c] + w[g]) / tgt))
        percore[c].append(g)
        loads[c] += w[g]
        ncnt[c] += 1
    asg = np.empty((NB, NCORES), np.int64)
    for c in range(NCORES):
        blocks = np.array(percore[c])
        o = np.argsort(-w[blocks].sum(axis=1))
        asg[:, c] = blocks[o]
    # interleave big/small positions
    riffle = []
    lo, hi = 0, NB - 1
    while lo <= hi:
        riffle.append(lo)
        if lo != hi:
            riffle.append(hi)
        lo += 1
        hi -= 1
    riffle = [r for r in riffle if r != NB - 1] + [NB - 1]
    asg = asg[np.array(riffle)]
    # greedy per-core realignment so per-stream cumsums track the mean
    cntb = w[asg]                                     # [NB, NCORES, NST]
    T = cntb.mean(axis=1).cumsum(axis=0)              # [NB, NST]
    mcw = np.array(MC, np.float64)
    asg_al = np.empty_like(asg)
    for c in range(NCORES):
        rem = list(asg[:, c])
        S = np.zeros(NST)
        for j in range(NB):
            best = None
            bi = None
            for i, g in enumerate(rem):
                cost = (np.abs(S + w[g] - T[j]) * mcw).sum()
                if best is None or cost < best:
                    best = cost
                    bi = i
            g = rem.pop(bi)
            S += w[g]
            asg_al[j, c] = g
    asg = asg_al
    pos_of = np.empty(NGB, np.int64)
    core_of = np.empty(NGB, np.int64)
    for j in range(NB):
        for c in range(NCORES):
            g = asg[j, c]
            pos_of[g] = j
            core_of[g] = c

    # per-core per-stream cumulative slot offsets by position
    S = np.zeros((NST, NCORES, NB + 1), np.int64)
    for c in range(NCORES):
        for j in range(NB):
            g = asg[j, c]
            for k in range(NST):
                S[k, c, j + 1] = S[k, c, j] + w[g, k]

    nch = [0] * NST
    lo_r = np.zeros((NST, NB), np.int64)
    hi_r = np.zeros((NST, NB), np.int64)
    for k in range(NST):
        nch[k] = max(1, int(-(-S[k, :, -1].max() // P)))
        lo_r[k] = np.min(S[k, :, :-1], axis=0) // P
        hi_r[k] = -(-np.max(S[k, :, 1:], axis=0) // P)
        hi_r[k] = np.minimum(hi_r[k], nch[k])
        hi_r[k] = np.maximum(hi_r[k], lo_r[k])
    pair_idx = [dict() for _ in range(NST)]
    npairs = [0] * NST
    for k in range(NST):
        n = 0
        for j in range(NB):
            for ch in range(int(lo_r[k][j]), int(hi_r[k][j])):
                pair_idx[k][(j, ch)] = n
                n += 1
        npairs[k] = n
    # member-column counts, padded to OHB builds
    nmem = [npairs[k] * MC[k] for k in range(NST)]
    nmem_p = [-(-nmem[k] // OHB) * OHB for k in range(NST)]

    iota_int = np.repeat(np.arange(P, dtype=np.float32), OHB)[None, :]
    iota_int = np.tile(iota_int, (P, 1)).astype(BF16)               # [128, 128*OHB]

    pad_idx = [SPLIT, B_ROWS, NPAIR_T - 1]
    in_maps = []
    for c in range(NCORES):
        # compact per-core dual-pair table: global q -> local row
        qs_all = np.concatenate([blk_slots[asg[j, c]][2][0] for j in range(NB)])
        uniq = np.unique(qs_all)
        assert len(uniq) <= NPAIR_T - 1, f"dual table overflow: {len(uniq)}"
        qmap = {int(q): i for i, q in enumerate(uniq.tolist())}
        tabP = np.zeros((NPAIR_T, 2 * D), BF16)
        r_arr = uniq // KPART
        s_arr = partner[r_arr, uniq % KPART]
        tabP[: len(uniq), :D] = h16[r_arr]
        tabP[: len(uniq), D:] = h16[s_arr]

        idxs = [np.full(nch[k] * P, pad_idx[k], np.int64) for k in range(NST)]
        dloc = [np.full((P, nmem_p[k]), PAD_DLOC, np.float32) for k in range(NST)]
        for j in range(NB):
            g = asg[j, c]
            for k in range(NST):
                sl0 = S[k, c, j]
                data = blk_slots[g][k]
                keys = data[0]
                n = len(keys)
                if n == 0:
                    continue
                sl = sl0 + np.arange(n)
                if k < 2:
                    idxs[k][sl] = keys
                else:
                    idxs[k][sl] = np.array([qmap[int(q)] for q in keys])
                pch = sl // P
                pid = np.array([pair_idx[k][(j, int(ch_))] for ch_ in pch])
                if k < 2:
                    dloc[k][sl % P, pid] = data[1] % P
                else:
                    dloc[k][sl % P, pid * 2] = data[1]
                    dloc[k][sl % P, pid * 2 + 1] = data[2]

        def idx_layout(v):
            w_ = v.astype(np.int16).reshape(-1, 16).T        # [16, S/16]
            return np.tile(w_, (8, 1)).copy()                # [128, S/16]

        hT = np.zeros((D, NPC), np.float32)
        rcp = np.zeros((1, NPC), np.float32)
        for j in range(NB):
            g = asg[j, c]
            lo2 = g * P
            hi2 = min(lo2 + P, N_NODES)
            if hi2 > lo2:
                hT[:, j * P : j * P + (hi2 - lo2)] = h[lo2:hi2].T
                rcp[0, j * P : j * P + (hi2 - lo2)] = recip[lo2:hi2]

        auxW = np.concatenate(
            [np.asarray(W1, np.float32), np.asarray(W2, np.float32)], axis=1
        ).astype(BF16)  # [128, 256]
        auxB = (np.asarray(b1, np.float32) + np.asarray(b2, np.float32))[:, None]

        in_maps.append(
            {
                "tabA": tabA,
                "tabB": tabB,
                "tabP": tabP,
                "idx0": idx_layout(idxs[0]),
                "idx1": idx_layout(idxs[1]),
                "idx2": idx_layout(idxs[2]),
                "dl0": dloc[0].astype(BF16),
                "dl1": dloc[1].astype(BF16),
                "dl2": dloc[2].astype(BF16),
                "hT": hT.astype(BF16),
                "rcp": rcp.astype(BF16),
                "ioI": iota_int,
                "auxW": auxW,
                "auxB": auxB.astype(np.float32),
            }
        )

    meta = dict(
        nch=nch,
        npairs=npairs,
        nmem=nmem,
        nmem_p=nmem_p,
        lo_r=lo_r,
        hi_r=hi_r,
        pair_idx=pair_idx,
        asg=asg,
    )
    return in_maps, meta


def _build(meta):
    import concourse.bacc as bacc
    import concourse.mybir as mybir
    import concourse.tile as tile

    nch = meta["nch"]
    nmem_p = meta["nmem_p"]
    lo_r, hi_r = meta["lo_r"], meta["hi_r"]
    pair_idx = meta["pair_idx"]
    f32 = mybir.dt.float32
    bf16 = mybir.dt.bfloat16

    nc = bacc.Bacc("TRN2", target_bir_lowering=False, debug=False, num_devices=NCORES)
    tabs_d = [
        nc.declare_dram_parameter("tabA", [SPLIT + 1, D], bf16, isOutput=False),
        nc.declare_dram_parameter("tabB", [B_ROWS + 1, D], bf16, isOutput=False),
        nc.declare_dram_parameter("tabP", [NPAIR_T, 2 * D], bf16, isOutput=False),
    ]
    idx_d = [
        nc.declare_dram_parameter(f"idx{k}", [P, nch[k] * 8], mybir.dt.int16, isOutput=False)
        for k in range(NST)
    ]
    dl_d = [
        nc.declare_dram_parameter(f"dl{k}", [P, nmem_p[k]], bf16, isOutput=False)
        for k in range(NST)
    ]
    hT = nc.declare_dram_parameter("hT", [D, NPC], bf16, isOutput=False)
    rcp = nc.declare_dram_parameter("rcp", [1, NPC], bf16, isOutput=False)
    ioI = nc.declare_dram_parameter("ioI", [P, P * OHB], bf16, isOutput=False)
    auxW = nc.declare_dram_parameter("auxW", [P, 256], bf16, isOutput=False)
    auxB = nc.declare_dram_parameter("auxB", [P, 1], f32, isOutput=False)
    outT = nc.declare_dram_parameter("outT", [D, NPC], bf16, isOutput=True)

    def batch_schedule(n, bch):
        sched = []
        pos = 0
        while n - pos > 0:
            step = min(bch, n - pos)
            sched.append((pos, step))
            pos += step
        if sched and sched[-1][1] > 8:
            st, ln = sched.pop()
            sched.append((st, ln - 8))
            sched.append((st + ln - 8, 8))
        return sched

    bsched = [batch_schedule(nch[k], BCH[k]) for k in range(NST)]
    bmap = {}
    for k in range(NST):
        for bi, (st, ln) in enumerate(bsched[k]):
            for ch in range(st, st + ln):
                bmap[(k, ch)] = bi

    with tile.TileContext(nc) as tc, ExitStack() as ctx:
        consts = ctx.enter_context(tc.tile_pool(name="consts", bufs=1))
        g_pools = [
            ctx.enter_context(tc.tile_pool(name="g0", bufs=5)),
            ctx.enter_context(tc.tile_pool(name="g1", bufs=4)),
            ctx.enter_context(tc.tile_pool(name="g2", bufs=4)),
        ]
        oh_pools = [
            ctx.enter_context(tc.tile_pool(name="oh0", bufs=8)),
            ctx.enter_context(tc.tile_pool(name="oh1", bufs=5)),
            ctx.enter_context(tc.tile_pool(name="oh2", bufs=4)),
        ]
        hN_pool = ctx.enter_context(tc.tile_pool(name="hNp", bufs=4))
        out_pool = ctx.enter_context(tc.tile_pool(name="outp", bufs=8))
        psA = ctx.enter_context(tc.tile_pool(name="psA", bufs=5, space="PSUM"))
        psO = ctx.enter_context(tc.tile_pool(name="psO", bufs=3, space="PSUM"))

        HEAD = 2 * 32
        idx_t = []
        dl_t = []
        for k in range(NST):
            it = consts.tile([P, nch[k] * 8], mybir.dt.int16, name=f"idx{k}")
            h8 = min(HEAD * 8, nch[k] * 8)
            nc.sync.dma_start(it[:, :h8], idx_d[k][:, :h8])
            if nch[k] * 8 > h8:
                nc.sync.dma_start(it[:, h8:], idx_d[k][:, h8:])
            idx_t.append(it)
            dt_ = consts.tile([P, nmem_p[k]], bf16, name=f"dl{k}")
            hd = min(HEAD, nmem_p[k])
            nc.sync.dma_start(dt_[:, :hd], dl_d[k][:, :hd])
            if nmem_p[k] > hd:
                nc.sync.dma_start(dt_[:, hd:], dl_d[k][:, hd:])
            dl_t.append(dt_)
        auxW_t = consts.tile([P, 256], bf16)
        nc.sync.dma_start(auxW_t[:], auxW[:])
        auxB_t = consts.tile([P, 1], f32)
        nc.sync.dma_start(auxB_t[:], auxB[:])
        ioI_t = consts.tile([P, P * OHB], bf16)
        nc.sync.dma_start(ioI_t[:], ioI[:])
        rcp_t = consts.tile([1, NPC], bf16)
        nc.sync.dma_start(rcp_t[:], rcp[:])
        hT_all = consts.tile([P, NPC], bf16)
        nc.sync.dma_start(hT_all[:], hT[:])
        w1_s = auxW_t[:, 0:128]
        w2_s = auxW_t[:, 128:256]
        bias_s = auxB_t[:, 0:1]

        rb_t = consts.tile([P, NPC], bf16)
        nc.gpsimd.partition_broadcast(rb_t[:], rcp_t[:])

        g_tiles = [dict() for _ in range(NST)]
        oh_state = [(-1, None) for _ in range(NST)]

        def chunk_ap(k, j):
            """AP of gathered slot-chunk j of stream k ([P, EW[k]] view)."""
            b = bmap[(k, j)]
            st, ln = bsched[k][b]
            off = j - st
            ew = EW[k]
            tile_ = g_tiles[k].get(b)
            if tile_ is None:
                gt = g_pools[k].tile([P, BCH[k] * ew], bf16, name=f"g{k}")
                ni = ln * P
                nc.gpsimd.dma_gather(
                    gt[:, : ln * ew].rearrange("p (c e) -> p c e", e=ew),
                    tabs_d[k][:, :],
                    idx_t[k][:, st * 8 : st * 8 + (ni // 16)],
                    ni,
                    ni,
                    ew,
                    single_packet=False,
                )
                g_tiles[k][b] = gt
                tile_ = gt
            return tile_[:, off * ew : (off + 1) * ew]

        def oh_ap(k, mcol):
            """AP of member column mcol of stream k (stride-OHB view)."""
            b, ci = divmod(mcol, OHB)
            cur, tile_ = oh_state[k]
            if b != cur:
                ot = oh_pools[k].tile([P, P * OHB], bf16, name=f"oh{k}")
                in1 = (
                    dl_t[k][:, b * OHB : (b + 1) * OHB]
                    .rearrange("p (x c) -> p x c", x=1)
                    .broadcast_to([P, P, OHB])
                )
                nc.vector.tensor_tensor(
                    out=ot[:].rearrange("p (d c) -> p d c", c=OHB),
                    in0=ioI_t[:].rearrange("p (d c) -> p d c", c=OHB),
                    in1=in1,
                    op=mybir.AluOpType.is_equal,
                )
                oh_state[k] = (b, ot)
                tile_ = ot
            return tile_[:].rearrange("p (d c) -> p c d", c=OHB)[:, ci, :]

        for j in range(NB):
            agg = psA.tile([P, P], f32)
            plan = []
            for k in range(NST):
                for ch in range(int(lo_r[k][j]), int(hi_r[k][j])):
                    pr = pair_idx[k][(j, ch)]
                    for i in range(MC[k]):
                        plan.append((k, ch, pr * MC[k] + i, i))
            assert plan, f"empty plan at position {j}"
            for n_, (k, ch, mcol, half) in enumerate(plan):
                g_ap = chunk_ap(k, ch)
                nc.tensor.matmul(
                    agg[:],
                    lhsT=g_ap[:, half * D : (half + 1) * D],
                    rhs=oh_ap(k, mcol),
                    start=(n_ == 0),
                    stop=(n_ == len(plan) - 1),
                )

            hN_t = hN_pool.tile([P, P], bf16)
            nc.vector.tensor_tensor(
                out=hN_t[:],
                in0=agg[:],
                in1=rb_t[:, j * P : (j + 1) * P],
                op=mybir.AluOpType.mult,
            )

            po = psO.tile([P, P], f32)
            nc.tensor.matmul(
                po[:], lhsT=w1_s, rhs=hT_all[:, j * P : (j + 1) * P], start=True, stop=False
            )
            nc.tensor.matmul(po[:], lhsT=w2_s, rhs=hN_t[:], start=False, stop=True)

            if j % 2 == 0:
                out_t2 = out_pool.tile([P, 2 * P], bf16)
            out_sl = out_t2[:, 0:P] if j % 2 == 0 else out_t2[:, P : 2 * P]
            nc.scalar.activation(
                out_sl, po[:], mybir.ActivationFunctionType.Identity, bias=bias_s
            )
            if j % 2 == 1:
                nc.sync.dma_start(outT[:, (j - 1) * P : (j + 1) * P], out_t2[:])
            elif j == NB - 1:
                nc.sync.dma_start(outT[:, j * P : (j + 1) * P], out_t2[:, 0:P])

    nc.finalize()
    return nc


def kernel(h, src, dst, W1, b1, W2, b2):
    from concourse.bass_utils import run_bass_kernel_spmd

    in_maps, meta = _prep(h, src, dst, W1, b1, W2, b2)
    nc = _build(meta)
    res = run_bass_kernel_spmd(nc, in_maps, list(range(NCORES))).results
    return _assemble(res, meta)


def _assemble(res, meta):
    asg = meta["asg"]
    out = np.zeros((NCORES * NPC, D), np.float32)
    for j in range(NB):
        for c in range(NCORES):
            g = asg[j, c]
            out[g * P : (g + 1) * P] = (
                res[c]["outT"][:, j * P : (j + 1) * P].astype(np.float32).T
            )
    return out[:N_NODES].astype(np.float32)


def _sim(h, src, dst, W1, b1, W2, b2):
    """Numpy simulation of the exact device program (bookkeeping check)."""
    in_maps, meta = _prep(h, src, dst, W1, b1, W2, b2)
    lo_r, hi_r = meta["lo_r"], meta["hi_r"]
    pair_idx = meta["pair_idx"]
    outs = []
    for c in range(NCORES):
        m = in_maps[c]
        tabs = [m["tabA"], m["tabB"], m["tabP"]]
        idxs = []
        dloc = []
        for k in range(NST):
            w_ = m[f"idx{k}"][:16]
            idxs.append(w_.T.reshape(-1))
            dloc.append(m[f"dl{k}"].astype(np.float32))
        W1b = m["auxW"][:, 0:128].astype(np.float32)
        W2b = m["auxW"][:, 128:256].astype(np.float32)
        rcpv = m["rcp"].astype(np.float32)[0]
        outT = np.zeros((D, NPC), np.float32)
        for j in range(NB):
            agg = np.zeros((P, P), np.float32)
            for k in range(NST):
                for ch in range(int(lo_r[k][j]), int(hi_r[k][j])):
                    pr = pair_idx[k][(j, ch)]
                    gi = idxs[k][ch * P : (ch + 1) * P].astype(np.int64)
                    Gfull = tabs[k][gi].astype(np.float32)   # [128, EW[k]]
                    for i in range(MC[k]):
                        G = Gfull[:, i * D : (i + 1) * D]
                        dv = dloc[k][:, (pr * MC[k] + i)]
                        O = (dv[:, None] == np.arange(P)[None, :]).astype(np.float32)
                        agg += G.T @ O
            hNj = (agg * rcpv[None, j * P : (j + 1) * P]).astype(BF16).astype(np.float32)
            hTj = m["hT"][:, j * P : (j + 1) * P].astype(np.float32)
            po = W1b.T @ hTj + W2b.T @ hNj
            outT[:, j * P : (j + 1) * P] = (po + m["auxB"]).astype(BF16)
        outs.append({"outT": outT.astype(BF16)})
    return _assemble(outs, meta)


if __name__ == "__main__":
    rng = np.random.default_rng(0)
    h = rng.standard_normal((N_NODES, D), dtype=np.float32)
    src = rng.integers(0, N_NODES, N_EDGES)
    dst = rng.integers(0, N_NODES, N_EDGES)
    W1 = rng.standard_normal((D, D), dtype=np.float32) * 0.1
    b1 = rng.standard_normal(D, dtype=np.float32) * 0.1
    W2 = rng.standard_normal((D, D), dtype=np.float32) * 0.1
    b2 = rng.standard_normal(D, dtype=np.float32) * 0.1

    msgs_sum = np.zeros((N_NODES, D), np.float32)
    np.add.at(msgs_sum, dst, h[src])
    deg = np.bincount(dst, minlength=N_NODES).astype(np.float32)
    hN = msgs_sum / np.maximum(deg, 1.0)[:, None]
    ref = h @ W1 + b1 + hN @ W2 + b2

    got = _sim(h, src, dst, W1, b1, W2, b2)
    err = np.linalg.norm(got - ref) / np.linalg.norm(ref)
    print("sim rel err (norm):", err)
    print("sim max abs err:", np.abs(got - ref).max())
    in_maps, meta = _prep(h, src, dst, W1, b1, W2, b2)
    print("nch:", meta["nch"], "npairs:", meta["npairs"])
    tot_slots = sum(meta["nch"][k] * 128 for k in range(NST))
    print("slot-descs/core:", tot_slots)


# revision 26
# speedup vs baseline: 1.3927x; 1.0727x over previous
"""GNN message-passing (copy_u -> segment mean -> two GEMMs) on 8 trn2 NeuronCores.

Strategy (1D dst partitioning + paired-descriptor gather):
  - Nodes are split into 392 blocks of 128; blocks are dealt to the 8 cores
    so per-position stream sizes match across cores. All in-edges of a node
    live on its owner core.
  - The dominant cost is the per-edge dma_gather of h rows (256B descriptors,
    22.76 ns each in the cost model; <512B transfers pay a 2x latency
    multiplier, so a 512B descriptor costs the SAME 22.76 ns). Exploit: build
    HBM "pair tables" whose row q is the concatenation (h[q] | h[partner[q]])
    for a host-chosen partner map. Two edges of the same dst block whose rows
    are (q, partner[q]) share ONE 512B descriptor ("dual" slots). Unmatched
    edges gather 256B rows from the plain tables as before ("single" slots).
  - partner[] is chosen to maximize co-occurrence of (q, partner[q]) inside
    dst blocks (argmax over the block-presence Gram matrix); a greedy
    per-block capacity matcher then converts co-occurrences into matched edge
    pairs (p ~ 0.31 of edges), cutting gather descriptors by ~p/2.
  - 4 gather streams per core: single-A/B and dual-A/B (A/B split at row/pair
    32767 because dma_gather indices are int16). Each stream is packed
    contiguously in (block, key) order; the 128-slot chunk grid does not align
    to block boundaries — a chunk on a boundary is consumed by both blocks,
    each with its own one-hot column(s) (foreign slots masked to PAD).
  - Aggregation per chunk: psum[feat,dst] += G.T @ O on the PE (bf16),
    O[e,d] = (dst_local[e]==d). Dual chunks run TWO matmuls (primary half /
    partner half of the 512B slot) with two one-hot columns. One DVE
    tensor_tensor(is_equal) builds OHB member-columns at once in an
    interleaved layout oh[e, d*OHB+c].
  - Mean division folded into the DVE PSUM evacuation; per block
    psum_out[dout,node] = W1.T @ hT_blk + W2.T @ hNT_blk with weights
    stationary; bias added during the ScalarE evacuation; output written
    feature-major bf16, transposed back on host.

Self-contained: only needs numpy (+scipy if present) and /opt/trn_rl_repo.
"""

import sys

if "/opt/trn_rl_repo" not in sys.path:
    sys.path.insert(0, "/opt/trn_rl_repo")

import numpy as np
from contextlib import ExitStack

import ml_dtypes

BF16 = ml_dtypes.bfloat16

N_NODES = 50000
N_EDGES = 800000
D = 128
P = 128
NCORES = 8
NB = 49                      # blocks per core
NPC = NB * P                 # nodes per core (6272)
SPLIT = 32767                # stream-A rows/pairs are [0, SPLIT); idx SPLIT = pad
B_ROWS = N_NODES - SPLIT     # 17233 data rows/pairs in the B tables
NST = 2                      # streams: 0 single, 1 dual (both per-core tables)
MC = [1, 2]                  # member (one-hot) columns / matmuls per chunk
EW = [D, 2 * D]              # gathered elements per slot
NSING_T = 32768              # per-core compact single table rows (pad = last)
NPAIR_T = 32768              # per-core compact dual-pair table rows (pad = last)
KPART = 5                    # partner candidates per row
OHB = 16                     # member columns per one-hot build
PAD_DLOC = 200.0             # dst_local sentinel -> one-hot column of zeros
BCH = [32, 16]               # chunks per gather batch, per stream


def _choose_partners(src, dst):
    """partner[r, :K] = rows maximizing #dst-blocks where both rows have edges."""
    gblk = (dst // P).astype(np.int64)
    ncb = (N_NODES + P - 1) // P
    pres = np.zeros((N_NODES, ncb), np.float32)
    pres[src, gblk] = 1.0
    partner = np.full((N_NODES, KPART), -1, np.int64)
    CH = 2048
    for lo in range(0, N_NODES, CH):
        hi = min(lo + CH, N_NODES)
        W = pres[lo:hi] @ pres.T                      # [CH, N] co-occurrence
        W[np.arange(hi - lo), np.arange(lo, hi)] = -1.0
        top = np.argpartition(W, -KPART, axis=1)[:, -KPART:]
        vals = np.take_along_axis(W, top, axis=1)
        order = np.argsort(-vals, axis=1)
        top = np.take_along_axis(top, order, axis=1)
        vals = np.take_along_axis(vals, order, axis=1)
        top[vals < 1.0] = -1                          # useless partners
        partner[lo:hi] = top
    return partner


def _match_blocks(src, dst, partner):
    """Per dst-block greedy capacity matching over K partner candidates.

    Returns (dual, single):
      dual:   per block b, list of (q, dst_r, dst_s) matched instances,
              q = r * KPART + ki (global pair id)
      single: per block b, (rows, dsts) arrays of unmatched edges
    """
    gblk = (dst // P).astype(np.int64)
    ncb = NB * NCORES
    order = np.argsort(gblk * N_NODES + src, kind="stable")
    bs = gblk[order]
    rs = src[order]
    ds = dst[order]
    starts = np.searchsorted(bs, np.arange(ncb))
    ends = np.searchsorted(bs, np.arange(ncb), side="right")
    dual = [[] for _ in range(ncb)]
    single = [None] * ncb
    for b in range(ncb):
        rows = rs[starts[b]:ends[b]]
        dsts = ds[starts[b]:ends[b]]
        vals, first = np.unique(rows, return_index=True)
        cnt = {}
        ptr = {}
        for v, f in zip(vals.tolist(), first.tolist()):
            c = np.searchsorted(rows, v, side="right") - f
            cnt[v] = c
            ptr[v] = f                                # next unconsumed edge of v
        used = np.zeros(len(rows), bool)
        dl = dual[b]

        def take(v):
            i = ptr[v]
            while used[i]:
                i += 1
            used[i] = True
            ptr[v] = i + 1
            return i

        for r in vals.tolist():
            if cnt.get(r, 0) <= 0:
                continue
            for ki in range(KPART):
                s = int(partner[r, ki])
                if s < 0 or s == r:
                    continue
                m = min(cnt[r], cnt.get(s, 0))
                if m <= 0:
                    continue
                q = r * KPART + ki
                for _ in range(m):
                    ir = take(r)
                    isx = take(s)
                    dl.append((q, int(dsts[ir]), int(dsts[isx])))
                cnt[r] -= m
                cnt[s] -= m
                if cnt[r] <= 0:
                    break
        keep = ~used
        single[b] = (rows[keep], dsts[keep])
    return dual, single


def _prep(h, src, dst, W1, b1, W2, b2):
    """Host-side scheduling. Returns (in_maps, meta)."""
    src = np.asarray(src).astype(np.int64)
    dst = np.asarray(dst).astype(np.int64)
    h = np.asarray(h, dtype=np.float32)

    deg = np.bincount(dst, minlength=N_NODES).astype(np.float32)
    recip = (1.0 / np.maximum(deg, 1.0)).astype(np.float32)

    partner = _choose_partners(src, dst)
    dual, single = _match_blocks(src, dst, partner)

    h16 = h.astype(BF16)
    # pair q = r*KPART+ki -> (h[r] | h[partner[r,ki]]); compacted per core below

    NGB = NB * NCORES                                # 392 grid blocks (last empty)

    # per-(global block, stream) slot lists: (key, dsts...) in sorted order
    blk_slots = [[None] * NST for _ in range(NGB)]
    w = np.zeros((NGB, NST), np.int64)
    for g in range(NGB):
        srows, sdsts = single[g]
        o = np.argsort(srows, kind="stable")
        blk_slots[g][0] = (srows[o], sdsts[o])
        dl = dual[g]
        if dl:
            q = np.array([x[0] for x in dl], np.int64)
            d1 = np.array([x[1] % P for x in dl], np.float32)
            d2 = np.array([x[2] % P for x in dl], np.float32)
            o = np.argsort(q, kind="stable")
            blk_slots[g][1] = (q[o], d1[o], d2[o])
        else:
            z = np.zeros(0, np.int64)
            zf = np.zeros(0, np.float32)
            blk_slots[g][1] = (z, zf, zf)
        for k in range(NST):
            w[g, k] = len(blk_slots[g][k][0])

    # balanced assignment of blocks to (core, position)
    tot = w.sum(axis=0).astype(np.float64)
    tgt = np.maximum(tot / NCORES, 1.0)
    blk_order = np.argsort(-w.sum(axis=1))
    loads = np.zeros((NCORES, NST))
    ncnt = np.zeros(NCORES, np.int64)
    percore = [[] for _ in range(NCORES)]
    for g in blk_order:
        cand = [c for c in range(NCORES) if ncnt[c] < NB]
        c = min(cand, key=lambda c: np.max((loads[c] + w[g]) / tgt))
        percore[c].append(g)
        loads[c] += w[g]
        ncnt[c] += 1
    asg = np.empty((NB, NCORES), np.int64)
    for c in range(NCORES):
        blocks = np.array(percore[c])
        o = np.argsort(-w[blocks].sum(axis=1))
        asg[:, c] = blocks[o]
    # interleave big/small positions
    riffle = []
    lo, hi = 0, NB - 1
    while lo <= hi:
        riffle.append(lo)
        if lo != hi:
            riffle.append(hi)
        lo += 1
        hi -= 1
    riffle = [r for r in riffle if r != NB - 1] + [NB - 1]
    asg = asg[np.array(riffle)]
    # greedy per-core realignment so per-stream cumsums track the mean
    cntb = w[asg]                                     # [NB, NCORES, NST]
    T = cntb.mean(axis=1).cumsum(axis=0)              # [NB, NST]
    mcw = np.array(MC, np.float64)
    asg_al = np.empty_like(asg)
    for c in range(NCORES):
        rem = list(asg[:, c])
        S = np.zeros(NST)
        for j in range(NB):
            best = None
            bi = None
            for i, g in enumerate(rem):
                cost = (np.abs(S + w[g] - T[j]) * mcw).sum()
                if best is None or cost < best:
                    best = cost
                    bi = i
            g = rem.pop(bi)
            S += w[g]
            asg_al[j, c] = g
    asg = asg_al

    # local-search: swap blocks within a core to shrink the shared chunk grid
    def grid_cost(Sar):
        lo = np.floor(Sar[:, :, :-1].min(axis=1) / P)
        hi = np.ceil(Sar[:, :, 1:].max(axis=1) / P)
        return float((np.maximum(hi - lo, 0) * mcw[:, None]).sum())

    Sar = np.zeros((NST, NCORES, NB + 1), np.float64)
    for c in range(NCORES):
        Sar[:, c, 1:] = np.cumsum(w[asg[:, c]].T, axis=1)
    cur = grid_cost(Sar)
    for _ in range(3):
        improved = False
        for c in range(NCORES):
            for j1 in range(NB - 1):
                for j2 in range(j1 + 1, NB):
                    g1, g2 = asg[j1, c], asg[j2, c]
                    if np.array_equal(w[g1], w[g2]):
                        continue
                    dwv = (w[g2] - w[g1]).astype(np.float64)
                    old = Sar[:, c, j1 + 1 : j2 + 1].copy()
                    Sar[:, c, j1 + 1 : j2 + 1] += dwv[:, None]
                    new = grid_cost(Sar)
                    if new < cur - 1e-9:
                        cur = new
                        asg[j1, c], asg[j2, c] = g2, g1
                        improved = True
                    else:
                        Sar[:, c, j1 + 1 : j2 + 1] = old
        if not improved:
            break

    pos_of = np.empty(NGB, np.int64)
    core_of = np.empty(NGB, np.int64)
    for j in range(NB):
        for c in range(NCORES):
            g = asg[j, c]
            pos_of[g] = j
            core_of[g] = c

    # per-core per-stream cumulative slot offsets by position
    S = np.zeros((NST, NCORES, NB + 1), np.int64)
    for c in range(NCORES):
        for j in range(NB):
            g = asg[j, c]
            for k in range(NST):
                S[k, c, j + 1] = S[k, c, j] + w[g, k]

    nch = [0] * NST
    lo_r = np.zeros((NST, NB), np.int64)
    hi_r = np.zeros((NST, NB), np.int64)
    for k in range(NST):
        nch[k] = max(1, int(-(-S[k, :, -1].max() // P)))
        lo_r[k] = np.min(S[k, :, :-1], axis=0) // P
        hi_r[k] = -(-np.max(S[k, :, 1:], axis=0) // P)
        hi_r[k] = np.minimum(hi_r[k], nch[k])
        hi_r[k] = np.maximum(hi_r[k], lo_r[k])
    pair_idx = [dict() for _ in range(NST)]
    npairs = [0] * NST
    for k in range(NST):
        n = 0
        for j in range(NB):
            for ch in range(int(lo_r[k][j]), int(hi_r[k][j])):
                pair_idx[k][(j, ch)] = n
                n += 1
        npairs[k] = n
    # member-column counts, padded to OHB builds
    nmem = [npairs[k] * MC[k] for k in range(NST)]
    nmem_p = [-(-nmem[k] // OHB) * OHB for k in range(NST)]

    iota_int = np.repeat(np.arange(P, dtype=np.float32), OHB)[None, :]
    iota_int = np.tile(iota_int, (P, 1)).astype(BF16)               # [128, 128*OHB]

    pad_idx = [NSING_T - 1, NPAIR_T - 1]
    in_maps = []
    for c in range(NCORES):
        # compact per-core dual-pair table: global q -> local row
        qs_all = np.concatenate([blk_slots[asg[j, c]][1][0] for j in range(NB)])
        uniq = np.unique(qs_all)
        assert len(uniq) <= NPAIR_T - 1, f"dual table overflow: {len(uniq)}"
        qmap = {int(q): i for i, q in enumerate(uniq.tolist())}
        tabP = np.zeros((NPAIR_T, 2 * D), BF16)
        r_arr = uniq // KPART
        s_arr = partner[r_arr, uniq % KPART]
        tabP[: len(uniq), :D] = h16[r_arr]
        tabP[: len(uniq), D:] = h16[s_arr]
        rs_all = np.concatenate([blk_slots[asg[j, c]][0][0] for j in range(NB)])
        uniqS = np.unique(rs_all)
        assert len(uniqS) <= NSING_T - 1, f"single table overflow: {len(uniqS)}"
        smap = {int(r): i for i, r in enumerate(uniqS.tolist())}
        tabS = np.zeros((NSING_T, D), BF16)
        tabS[: len(uniqS)] = h16[uniqS]

        idxs = [np.full(nch[k] * P, pad_idx[k], np.int64) for k in range(NST)]
        dloc = [np.full((P, nmem_p[k]), PAD_DLOC, np.float32) for k in range(NST)]
        for j in range(NB):
            g = asg[j, c]
            for k in range(NST):
                sl0 = S[k, c, j]
                data = blk_slots[g][k]
                keys = data[0]
                n = len(keys)
                if n == 0:
                    continue
                sl = sl0 + np.arange(n)
                if k == 0:
                    idxs[k][sl] = np.array([smap[int(r)] for r in keys])
                else:
                    idxs[k][sl] = np.array([qmap[int(q)] for q in keys])
                pch = sl // P
                pid = np.array([pair_idx[k][(j, int(ch_))] for ch_ in pch])
                if k == 0:
                    dloc[k][sl % P, pid] = data[1] % P
                else:
                    dloc[k][sl % P, pid * 2] = data[1]
                    dloc[k][sl % P, pid * 2 + 1] = data[2]

        def idx_layout(v):
            w_ = v.astype(np.int16).reshape(-1, 16).T        # [16, S/16]
            return np.tile(w_, (8, 1)).copy()                # [128, S/16]

        hT = np.zeros((D, NPC), np.float32)
        rcp = np.zeros((1, NPC), np.float32)
        for j in range(NB):
            g = asg[j, c]
            lo2 = g * P
            hi2 = min(lo2 + P, N_NODES)
            if hi2 > lo2:
                hT[:, j * P : j * P + (hi2 - lo2)] = h[lo2:hi2].T
                rcp[0, j * P : j * P + (hi2 - lo2)] = recip[lo2:hi2]

        auxW = np.concatenate(
            [np.asarray(W1, np.float32), np.asarray(W2, np.float32)], axis=1
        ).astype(BF16)  # [128, 256]
        auxB = (np.asarray(b1, np.float32) + np.asarray(b2, np.float32))[:, None]

        in_maps.append(
            {
                "tabS": tabS,
                "tabP": tabP,
                "idx0": idx_layout(idxs[0]),
                "idx1": idx_layout(idxs[1]),
                "dl0": dloc[0].astype(BF16),
                "dl1": dloc[1].astype(BF16),
                "hT": hT.astype(BF16),
                "rcp": rcp.astype(BF16),
                "ioI": iota_int,
                "auxW": auxW,
                "auxB": auxB.astype(np.float32),
            }
        )

    meta = dict(
        nch=nch,
        npairs=npairs,
        nmem=nmem,
        nmem_p=nmem_p,
        lo_r=lo_r,
        hi_r=hi_r,
        pair_idx=pair_idx,
        asg=asg,
    )
    return in_maps, meta


def _build(meta):
    import concourse.bacc as bacc
    import concourse.mybir as mybir
    import concourse.tile as tile

    nch = meta["nch"]
    nmem_p = meta["nmem_p"]
    lo_r, hi_r = meta["lo_r"], meta["hi_r"]
    pair_idx = meta["pair_idx"]
    f32 = mybir.dt.float32
    bf16 = mybir.dt.bfloat16

    nc = bacc.Bacc("TRN2", target_bir_lowering=False, debug=False, num_devices=NCORES)
    tabs_d = [
        nc.declare_dram_parameter("tabS", [NSING_T, D], bf16, isOutput=False),
        nc.declare_dram_parameter("tabP", [NPAIR_T, 2 * D], bf16, isOutput=False),
    ]
    idx_d = [
        nc.declare_dram_parameter(f"idx{k}", [P, nch[k] * 8], mybir.dt.int16, isOutput=False)
        for k in range(NST)
    ]
    dl_d = [
        nc.declare_dram_parameter(f"dl{k}", [P, nmem_p[k]], bf16, isOutput=False)
        for k in range(NST)
    ]
    hT = nc.declare_dram_parameter("hT", [D, NPC], bf16, isOutput=False)
    rcp = nc.declare_dram_parameter("rcp", [1, NPC], bf16, isOutput=False)
    ioI = nc.declare_dram_parameter("ioI", [P, P * OHB], bf16, isOutput=False)
    auxW = nc.declare_dram_parameter("auxW", [P, 256], bf16, isOutput=False)
    auxB = nc.declare_dram_parameter("auxB", [P, 1], f32, isOutput=False)
    outT = nc.declare_dram_parameter("outT", [D, NPC], bf16, isOutput=True)

    def batch_schedule(n, bch):
        sched = []
        pos = 0
        while n - pos > 0:
            step = min(bch, n - pos)
            sched.append((pos, step))
            pos += step
        if sched and sched[-1][1] > 8:
            st, ln = sched.pop()
            sched.append((st, ln - 8))
            sched.append((st + ln - 8, 8))
        return sched

    bsched = [batch_schedule(nch[k], BCH[k]) for k in range(NST)]
    bmap = {}
    for k in range(NST):
        for bi, (st, ln) in enumerate(bsched[k]):
            for ch in range(st, st + ln):
                bmap[(k, ch)] = bi

    with tile.TileContext(nc) as tc, ExitStack() as ctx:
        consts = ctx.enter_context(tc.tile_pool(name="consts", bufs=1))
        g_pools = [
            ctx.enter_context(tc.tile_pool(name="g0", bufs=5)),
            ctx.enter_context(tc.tile_pool(name="g1", bufs=4)),
        ]
        oh_pools = [
            ctx.enter_context(tc.tile_pool(name="oh0", bufs=6)),
            ctx.enter_context(tc.tile_pool(name="oh1", bufs=4)),
        ]
        hN_pool = ctx.enter_context(tc.tile_pool(name="hNp", bufs=4))
        out_pool = ctx.enter_context(tc.tile_pool(name="outp", bufs=8))
        psA = ctx.enter_context(tc.tile_pool(name="psA", bufs=5, space="PSUM"))
        psO = ctx.enter_context(tc.tile_pool(name="psO", bufs=3, space="PSUM"))

        HEAD = 2 * 32
        idx_t = []
        dl_t = []
        for k in range(NST):
            it = consts.tile([P, nch[k] * 8], mybir.dt.int16, name=f"idx{k}")
            h8 = min(HEAD * 8, nch[k] * 8)
            nc.sync.dma_start(it[:, :h8], idx_d[k][:, :h8])
            if nch[k] * 8 > h8:
                nc.sync.dma_start(it[:, h8:], idx_d[k][:, h8:])
            idx_t.append(it)
            dt_ = consts.tile([P, nmem_p[k]], bf16, name=f"dl{k}")
            hd = min(HEAD, nmem_p[k])
            nc.sync.dma_start(dt_[:, :hd], dl_d[k][:, :hd])
            if nmem_p[k] > hd:
                nc.sync.dma_start(dt_[:, hd:], dl_d[k][:, hd:])
            dl_t.append(dt_)
        auxW_t = consts.tile([P, 256], bf16)
        nc.sync.dma_start(auxW_t[:], auxW[:])
        auxB_t = consts.tile([P, 1], f32)
        nc.sync.dma_start(auxB_t[:], auxB[:])
        ioI_t = consts.tile([P, P * OHB], bf16)
        nc.sync.dma_start(ioI_t[:], ioI[:])
        rcp_t = consts.tile([1, NPC], bf16)
        nc.sync.dma_start(rcp_t[:], rcp[:])
        hT_all = consts.tile([P, NPC], bf16)
        nc.sync.dma_start(hT_all[:], hT[:])
        w1_s = auxW_t[:, 0:128]
        w2_s = auxW_t[:, 128:256]
        bias_s = auxB_t[:, 0:1]

        rb_t = consts.tile([P, NPC], bf16)
        nc.gpsimd.partition_broadcast(rb_t[:], rcp_t[:])

        g_tiles = [dict() for _ in range(NST)]
        oh_state = [(-1, None) for _ in range(NST)]

        def chunk_ap(k, j):
            """AP of gathered slot-chunk j of stream k ([P, EW[k]] view)."""
            b = bmap[(k, j)]
            st, ln = bsched[k][b]
            off = j - st
            ew = EW[k]
            tile_ = g_tiles[k].get(b)
            if tile_ is None:
                gt = g_pools[k].tile([P, BCH[k] * ew], bf16, name=f"g{k}")
                ni = ln * P
                nc.gpsimd.dma_gather(
                    gt[:, : ln * ew].rearrange("p (c e) -> p c e", e=ew),
                    tabs_d[k][:, :],
                    idx_t[k][:, st * 8 : st * 8 + (ni // 16)],
                    ni,
                    ni,
                    ew,
                    single_packet=False,
                )
                g_tiles[k][b] = gt
                tile_ = gt
            return tile_[:, off * ew : (off + 1) * ew]

        def oh_ap(k, mcol):
            """AP of member column mcol of stream k (stride-OHB view)."""
            b, ci = divmod(mcol, OHB)
            cur, tile_ = oh_state[k]
            if b != cur:
                ot = oh_pools[k].tile([P, P * OHB], bf16, name=f"oh{k}")
                in1 = (
                    dl_t[k][:, b * OHB : (b + 1) * OHB]
                    .rearrange("p (x c) -> p x c", x=1)
                    .broadcast_to([P, P, OHB])
                )
                nc.vector.tensor_tensor(
                    out=ot[:].rearrange("p (d c) -> p d c", c=OHB),
                    in0=ioI_t[:].rearrange("p (d c) -> p d c", c=OHB),
                    in1=in1,
                    op=mybir.AluOpType.is_equal,
                )
                oh_state[k] = (b, ot)
                tile_ = ot
            return tile_[:].rearrange("p (d c) -> p c d", c=OHB)[:, ci, :]

        for j in range(NB):
            agg = psA.tile([P, P], f32)
            plan = []
            for k in range(NST):
                for ch in range(int(lo_r[k][j]), int(hi_r[k][j])):
                    pr = pair_idx[k][(j, ch)]
                    for i in range(MC[k]):
                        plan.append((k, ch, pr * MC[k] + i, i))
            assert plan, f"empty plan at position {j}"
            for n_, (k, ch, mcol, half) in enumerate(plan):
                g_ap = chunk_ap(k, ch)
                nc.tensor.matmul(
                    agg[:],
                    lhsT=g_ap[:, half * D : (half + 1) * D],
                    rhs=oh_ap(k, mcol),
                    start=(n_ == 0),
                    stop=(n_ == len(plan) - 1),
                )

            hN_t = hN_pool.tile([P, P], bf16)
            nc.vector.tensor_tensor(
                out=hN_t[:],
                in0=agg[:],
                in1=rb_t[:, j * P : (j + 1) * P],
                op=mybir.AluOpType.mult,
            )

            po = psO.tile([P, P], f32)
            nc.tensor.matmul(
                po[:], lhsT=w1_s, rhs=hT_all[:, j * P : (j + 1) * P], start=True, stop=False
            )
            nc.tensor.matmul(po[:], lhsT=w2_s, rhs=hN_t[:], start=False, stop=True)

            if j % 2 == 0:
                out_t2 = out_pool.tile([P, 2 * P], bf16)
            out_sl = out_t2[:, 0:P] if j % 2 == 0 else out_t2[:, P : 2 * P]
            nc.scalar.activation(
                out_sl, po[:], mybir.ActivationFunctionType.Identity, bias=bias_s
            )
            if j % 2 == 1:
                nc.sync.dma_start(outT[:, (j - 1) * P : (j + 1) * P], out_t2[:])
            elif j == NB - 1:
                nc.sync.dma_start(outT[:, j * P : (j + 1) * P], out_t2[:, 0:P])

    nc.finalize()
    return nc


def kernel(h, src, dst, W1, b1, W2, b2):
    from concourse.bass_utils import run_bass_kernel_spmd

    in_maps, meta = _prep(h, src, dst, W1, b1, W2, b2)
    nc = _build(meta)
    res = run_bass_kernel_spmd(nc, in_maps, list(range(NCORES))).results
    return _assemble(res, meta)


def _assemble(res, meta):
    asg = meta["asg"]
    out = np.zeros((NCORES * NPC, D), np.float32)
    for j in range(NB):
        for c in range(NCORES):
            g = asg[j, c]
            out[g * P : (g + 1) * P] = (
                res[c]["outT"][:, j * P : (j + 1) * P].astype(np.float32).T
            )
    return out[:N_NODES].astype(np.float32)


def _sim(h, src, dst, W1, b1, W2, b2):
    """Numpy simulation of the exact device program (bookkeeping check)."""
    in_maps, meta = _prep(h, src, dst, W1, b1, W2, b2)
    lo_r, hi_r = meta["lo_r"], meta["hi_r"]
    pair_idx = meta["pair_idx"]
    outs = []
    for c in range(NCORES):
        m = in_maps[c]
        tabs = [m["tabS"], m["tabP"]]
        idxs = []
        dloc = []
        for k in range(NST):
            w_ = m[f"idx{k}"][:16]
            idxs.append(w_.T.reshape(-1))
            dloc.append(m[f"dl{k}"].astype(np.float32))
        W1b = m["auxW"][:, 0:128].astype(np.float32)
        W2b = m["auxW"][:, 128:256].astype(np.float32)
        rcpv = m["rcp"].astype(np.float32)[0]
        outT = np.zeros((D, NPC), np.float32)
        for j in range(NB):
            agg = np.zeros((P, P), np.float32)
            for k in range(NST):
                for ch in range(int(lo_r[k][j]), int(hi_r[k][j])):
                    pr = pair_idx[k][(j, ch)]
                    gi = idxs[k][ch * P : (ch + 1) * P].astype(np.int64)
                    Gfull = tabs[k][gi].astype(np.float32)   # [128, EW[k]]
                    for i in range(MC[k]):
                        G = Gfull[:, i * D : (i + 1) * D]
                        dv = dloc[k][:, (pr * MC[k] + i)]
                        O = (dv[:, None] == np.arange(P)[None, :]).astype(np.float32)
                        agg += G.T @ O
            hNj = (agg * rcpv[None, j * P : (j + 1) * P]).astype(BF16).astype(np.float32)
            hTj = m["hT"][:, j * P : (j + 1) * P].astype(np.float32)
            po = W1b.T @ hTj + W2b.T @ hNj
            outT[:, j * P : (j + 1) * P] = (po + m["auxB"]).astype(BF16)
        outs.append({"outT": outT.astype(BF16)})
    return _assemble(outs, meta)


if __name__ == "__main__":
    rng = np.random.default_rng(0)
    h = rng.standard_normal((N_NODES, D), dtype=np.float32)
    src = rng.integers(0, N_NODES, N_EDGES)
    dst = rng.integers(0, N_NODES, N_EDGES)
    W1 = rng.standard_normal((D, D), dtype=np.float32) * 0.1
    b1 = rng.standard_normal(D, dtype=np.float32) * 0.1
    W2 = rng.standard_normal((D, D), dtype=np.float32) * 0.1
    b2 = rng.standard_normal(D, dtype=np.float32) * 0.1

    msgs_sum = np.zeros((N_NODES, D), np.float32)
    np.add.at(msgs_sum, dst, h[src])
    deg = np.bincount(dst, minlength=N_NODES).astype(np.float32)
    hN = msgs_sum / np.maximum(deg, 1.0)[:, None]
    ref = h @ W1 + b1 + hN @ W2 + b2

    got = _sim(h, src, dst, W1, b1, W2, b2)
    err = np.linalg.norm(got - ref) / np.linalg.norm(ref)
    print("sim rel err (norm):", err)
    print("sim max abs err:", np.abs(got - ref).max())
    in_maps, meta = _prep(h, src, dst, W1, b1, W2, b2)
    print("nch:", meta["nch"], "npairs:", meta["npairs"])
    tot_slots = sum(meta["nch"][k] * 128 for k in range(NST))
    print("slot-descs/core:", tot_slots)


# revision 28
# speedup vs baseline: 1.4557x; 1.0452x over previous
"""GNN message-passing (copy_u -> segment mean -> two GEMMs) on 8 trn2 NeuronCores.

Strategy (1D dst partitioning + paired-descriptor gather):
  - Nodes are split into 392 blocks of 128; blocks are dealt to the 8 cores
    so per-position stream sizes match across cores. All in-edges of a node
    live on its owner core.
  - The dominant cost is the per-edge dma_gather of h rows (256B descriptors,
    22.76 ns each in the cost model; <512B transfers pay a 2x latency
    multiplier, so a 512B descriptor costs the SAME 22.76 ns). Exploit: build
    HBM "pair tables" whose row q is the concatenation (h[q] | h[partner[q]])
    for a host-chosen partner map. Two edges of the same dst block whose rows
    are (q, partner[q]) share ONE 512B descriptor ("dual" slots). Unmatched
    edges gather 256B rows from the plain tables as before ("single" slots).
  - partner[] is chosen to maximize co-occurrence of (q, partner[q]) inside
    dst blocks (argmax over the block-presence Gram matrix); a greedy
    per-block capacity matcher then converts co-occurrences into matched edge
    pairs (p ~ 0.31 of edges), cutting gather descriptors by ~p/2.
  - 4 gather streams per core: single-A/B and dual-A/B (A/B split at row/pair
    32767 because dma_gather indices are int16). Each stream is packed
    contiguously in (block, key) order; the 128-slot chunk grid does not align
    to block boundaries — a chunk on a boundary is consumed by both blocks,
    each with its own one-hot column(s) (foreign slots masked to PAD).
  - Aggregation per chunk: psum[feat,dst] += G.T @ O on the PE (bf16),
    O[e,d] = (dst_local[e]==d). Dual chunks run TWO matmuls (primary half /
    partner half of the 512B slot) with two one-hot columns. One DVE
    tensor_tensor(is_equal) builds OHB member-columns at once in an
    interleaved layout oh[e, d*OHB+c].
  - Mean division folded into the DVE PSUM evacuation; per block
    psum_out[dout,node] = W1.T @ hT_blk + W2.T @ hNT_blk with weights
    stationary; bias added during the ScalarE evacuation; output written
    feature-major bf16, transposed back on host.

Self-contained: only needs numpy (+scipy if present) and /opt/trn_rl_repo.
"""

import sys

if "/opt/trn_rl_repo" not in sys.path:
    sys.path.insert(0, "/opt/trn_rl_repo")

import numpy as np
from contextlib import ExitStack

import ml_dtypes

BF16 = ml_dtypes.bfloat16

N_NODES = 50000
N_EDGES = 800000
D = 128
P = 128
NCORES = 8
NB = 49                      # blocks per core
NPC = NB * P                 # nodes per core (6272)
SPLIT = 32767                # stream-A rows/pairs are [0, SPLIT); idx SPLIT = pad
B_ROWS = N_NODES - SPLIT     # 17233 data rows/pairs in the B tables
NST = 3                      # streams: 0 single, 1 dual-A (r<RSPLIT), 2 dual-B
MC = [1, 2, 2]               # member (one-hot) columns / matmuls per chunk
EW = [D, 2 * D, 2 * D]       # gathered elements per slot
RSPLIT = 25000               # dual stream split by primary row id
NSING_T = 32768              # per-core compact single table rows (pad = last)
NPAIR_T = 32768              # per-core compact dual-pair table rows (pad = last)
KPART = 5                    # partner candidates per row (round 1)
KPART2 = 5                   # partner candidates per row (residual round 2)
KTOT = KPART + KPART2        # pair-id stride per row
OHB = 16                     # member columns per one-hot build
PAD_DLOC = 200.0             # dst_local sentinel -> one-hot column of zeros
BCH = [32, 16, 16]           # chunks per gather batch, per stream


def _choose_partners(src, dst, k):
    """partner[r, :k] = rows maximizing #dst-blocks where both rows have edges."""
    gblk = (dst // P).astype(np.int64)
    ncb = (N_NODES + P - 1) // P
    pres = np.zeros((N_NODES, ncb), np.float32)
    pres[src, gblk] = 1.0
    partner = np.full((N_NODES, k), -1, np.int64)
    CH = 2048
    for lo in range(0, N_NODES, CH):
        hi = min(lo + CH, N_NODES)
        W = pres[lo:hi] @ pres.T                      # [CH, N] co-occurrence
        W[np.arange(hi - lo), np.arange(lo, hi)] = -1.0
        top = np.argpartition(W, -k, axis=1)[:, -k:]
        vals = np.take_along_axis(W, top, axis=1)
        order = np.argsort(-vals, axis=1)
        top = np.take_along_axis(top, order, axis=1)
        vals = np.take_along_axis(vals, order, axis=1)
        top[vals < 1.0] = -1                          # useless partners
        partner[lo:hi] = top
    return partner


def _match_blocks(src, dst, partner, koff):
    """Per dst-block greedy capacity matching over partner candidates.

    Returns (dual, single):
      dual:   per block b, list of (q, dst_r, dst_s) matched instances,
              q = r * KTOT + koff + ki (global pair id)
      single: per block b, (rows, dsts) arrays of unmatched edges
    """
    gblk = (dst // P).astype(np.int64)
    ncb = NB * NCORES
    order = np.argsort(gblk * N_NODES + src, kind="stable")
    bs = gblk[order]
    rs = src[order]
    ds = dst[order]
    starts = np.searchsorted(bs, np.arange(ncb))
    ends = np.searchsorted(bs, np.arange(ncb), side="right")
    dual = [[] for _ in range(ncb)]
    single = [None] * ncb
    for b in range(ncb):
        rows = rs[starts[b]:ends[b]]
        dsts = ds[starts[b]:ends[b]]
        vals, first = np.unique(rows, return_index=True)
        cnt = {}
        ptr = {}
        for v, f in zip(vals.tolist(), first.tolist()):
            c = np.searchsorted(rows, v, side="right") - f
            cnt[v] = c
            ptr[v] = f                                # next unconsumed edge of v
        used = np.zeros(len(rows), bool)
        dl = dual[b]

        def take(v):
            i = ptr[v]
            while used[i]:
                i += 1
            used[i] = True
            ptr[v] = i + 1
            return i

        nk = partner.shape[1]
        for r in vals[np.argsort(-first * 0 - np.array([cnt[v] for v in vals.tolist()]))].tolist():
            if cnt.get(r, 0) <= 0:
                continue
            for ki in range(nk):
                s = int(partner[r, ki])
                if s < 0 or s == r:
                    continue
                m = min(cnt[r], cnt.get(s, 0))
                if m <= 0:
                    continue
                q = r * KTOT + koff + ki
                for _ in range(m):
                    ir = take(r)
                    isx = take(s)
                    dl.append((q, int(dsts[ir]), int(dsts[isx])))
                cnt[r] -= m
                cnt[s] -= m
                if cnt[r] <= 0:
                    break
        keep = ~used
        single[b] = (rows[keep], dsts[keep])
    return dual, single


def _prep(h, src, dst, W1, b1, W2, b2):
    """Host-side scheduling. Returns (in_maps, meta)."""
    src = np.asarray(src).astype(np.int64)
    dst = np.asarray(dst).astype(np.int64)
    h = np.asarray(h, dtype=np.float32)

    deg = np.bincount(dst, minlength=N_NODES).astype(np.float32)
    recip = (1.0 / np.maximum(deg, 1.0)).astype(np.float32)

    partner1 = _choose_partners(src, dst, KPART)
    dual, single = _match_blocks(src, dst, partner1, 0)
    srcR = np.concatenate([s[0] for s in single])
    dstR = np.concatenate([s[1] for s in single])
    partner2 = _choose_partners(srcR, dstR, KPART2)
    dual2, single = _match_blocks(srcR, dstR, partner2, KPART)
    for b in range(len(dual)):
        dual[b] = dual[b] + dual2[b]
    partner = np.concatenate([partner1, partner2], axis=1)   # [N, KTOT]

    h16 = h.astype(BF16)
    # pair q = r*KPART+ki -> (h[r] | h[partner[r,ki]]); compacted per core below

    NGB = NB * NCORES                                # 392 grid blocks (last empty)

    # per-(global block, stream) slot lists: (key, dsts...) in sorted order
    blk_slots = [[None] * NST for _ in range(NGB)]
    w = np.zeros((NGB, NST), np.int64)
    for g in range(NGB):
        srows, sdsts = single[g]
        o = np.argsort(srows, kind="stable")
        blk_slots[g][0] = (srows[o], sdsts[o])
        dl = dual[g]
        if dl:
            q = np.array([x[0] for x in dl], np.int64)
            d1 = np.array([x[1] % P for x in dl], np.float32)
            d2 = np.array([x[2] % P for x in dl], np.float32)
            o = np.argsort(q, kind="stable")
            q, d1, d2 = q[o], d1[o], d2[o]
            mA = (q // KTOT) < RSPLIT
            blk_slots[g][1] = (q[mA], d1[mA], d2[mA])
            blk_slots[g][2] = (q[~mA], d1[~mA], d2[~mA])
        else:
            z = np.zeros(0, np.int64)
            zf = np.zeros(0, np.float32)
            blk_slots[g][1] = (z, zf, zf)
            blk_slots[g][2] = (z, zf, zf)
        for k in range(NST):
            w[g, k] = len(blk_slots[g][k][0])

    # balanced assignment of blocks to (core, position)
    tot = w.sum(axis=0).astype(np.float64)
    tgt = np.maximum(tot / NCORES, 1.0)
    blk_order = np.argsort(-w.sum(axis=1))
    loads = np.zeros((NCORES, NST))
    ncnt = np.zeros(NCORES, np.int64)
    percore = [[] for _ in range(NCORES)]
    for g in blk_order:
        cand = [c for c in range(NCORES) if ncnt[c] < NB]
        c = min(cand, key=lambda c: np.max((loads[c] + w[g]) / tgt))
        percore[c].append(g)
        loads[c] += w[g]
        ncnt[c] += 1
    asg = np.empty((NB, NCORES), np.int64)
    for c in range(NCORES):
        blocks = np.array(percore[c])
        o = np.argsort(-w[blocks].sum(axis=1))
        asg[:, c] = blocks[o]
    # interleave big/small positions
    riffle = []
    lo, hi = 0, NB - 1
    while lo <= hi:
        riffle.append(lo)
        if lo != hi:
            riffle.append(hi)
        lo += 1
        hi -= 1
    riffle = [r for r in riffle if r != NB - 1] + [NB - 1]
    asg = asg[np.array(riffle)]
    # greedy per-core realignment so per-stream cumsums track the mean
    cntb = w[asg]                                     # [NB, NCORES, NST]
    T = cntb.mean(axis=1).cumsum(axis=0)              # [NB, NST]
    mcw = np.array(MC, np.float64)
    asg_al = np.empty_like(asg)
    for c in range(NCORES):
        rem = list(asg[:, c])
        S = np.zeros(NST)
        for j in range(NB):
            best = None
            bi = None
            for i, g in enumerate(rem):
                cost = (np.abs(S + w[g] - T[j]) * mcw).sum()
                if best is None or cost < best:
                    best = cost
                    bi = i
            g = rem.pop(bi)
            S += w[g]
            asg_al[j, c] = g
    asg = asg_al

    # local-search: swap blocks within a core to shrink the shared chunk grid
    def grid_cost(Sar):
        lo = np.floor(Sar[:, :, :-1].min(axis=1) / P)
        hi = np.ceil(Sar[:, :, 1:].max(axis=1) / P)
        return float((np.maximum(hi - lo, 0) * mcw[:, None]).sum())

    Sar = np.zeros((NST, NCORES, NB + 1), np.float64)
    for c in range(NCORES):
        Sar[:, c, 1:] = np.cumsum(w[asg[:, c]].T, axis=1)
    cur = grid_cost(Sar)
    for _ in range(3):
        improved = False
        for c in range(NCORES):
            for j1 in range(NB - 1):
                for j2 in range(j1 + 1, NB):
                    g1, g2 = asg[j1, c], asg[j2, c]
                    if np.array_equal(w[g1], w[g2]):
                        continue
                    dwv = (w[g2] - w[g1]).astype(np.float64)
                    old = Sar[:, c, j1 + 1 : j2 + 1].copy()
                    Sar[:, c, j1 + 1 : j2 + 1] += dwv[:, None]
                    new = grid_cost(Sar)
                    if new < cur - 1e-9:
                        cur = new
                        asg[j1, c], asg[j2, c] = g2, g1
                        improved = True
                    else:
                        Sar[:, c, j1 + 1 : j2 + 1] = old
        if not improved:
            break

    pos_of = np.empty(NGB, np.int64)
    core_of = np.empty(NGB, np.int64)
    for j in range(NB):
        for c in range(NCORES):
            g = asg[j, c]
            pos_of[g] = j
            core_of[g] = c

    # per-core per-stream cumulative slot offsets by position
    S = np.zeros((NST, NCORES, NB + 1), np.int64)
    for c in range(NCORES):
        for j in range(NB):
            g = asg[j, c]
            for k in range(NST):
                S[k, c, j + 1] = S[k, c, j] + w[g, k]

    nch = [0] * NST
    lo_r = np.zeros((NST, NB), np.int64)
    hi_r = np.zeros((NST, NB), np.int64)
    for k in range(NST):
        nch[k] = max(1, int(-(-S[k, :, -1].max() // P)))
        lo_r[k] = np.min(S[k, :, :-1], axis=0) // P
        hi_r[k] = -(-np.max(S[k, :, 1:], axis=0) // P)
        hi_r[k] = np.minimum(hi_r[k], nch[k])
        hi_r[k] = np.maximum(hi_r[k], lo_r[k])
    pair_idx = [dict() for _ in range(NST)]
    npairs = [0] * NST
    for k in range(NST):
        n = 0
        for j in range(NB):
            for ch in range(int(lo_r[k][j]), int(hi_r[k][j])):
                pair_idx[k][(j, ch)] = n
                n += 1
        npairs[k] = n
    # member-column counts, padded to OHB builds
    nmem = [npairs[k] * MC[k] for k in range(NST)]
    nmem_p = [-(-nmem[k] // OHB) * OHB for k in range(NST)]

    iota_int = np.repeat(np.arange(P, dtype=np.float32), OHB)[None, :]
    iota_int = np.tile(iota_int, (P, 1)).astype(BF16)               # [128, 128*OHB]

    pad_idx = [NSING_T - 1, NPAIR_T - 1, NPAIR_T - 1]
    in_maps = []
    for c in range(NCORES):
        # compact per-core dual-pair table: global q -> local row
        tabPs = []
        qmaps = []
        for st in (1, 2):
            qs_all = np.concatenate([blk_slots[asg[j, c]][st][0] for j in range(NB)])
            uniq = np.unique(qs_all)
            assert len(uniq) <= NPAIR_T - 1, f"dual table overflow: {len(uniq)}"
            qmaps.append({int(q): i for i, q in enumerate(uniq.tolist())})
            tabP = np.zeros((NPAIR_T, 2 * D), BF16)
            r_arr = uniq // KTOT
            s_arr = partner[r_arr, uniq % KTOT]
            tabP[: len(uniq), :D] = h16[r_arr]
            tabP[: len(uniq), D:] = h16[s_arr]
            tabPs.append(tabP)
        rs_all = np.concatenate([blk_slots[asg[j, c]][0][0] for j in range(NB)])
        uniqS = np.unique(rs_all)
        assert len(uniqS) <= NSING_T - 1, f"single table overflow: {len(uniqS)}"
        smap = {int(r): i for i, r in enumerate(uniqS.tolist())}
        tabS = np.zeros((NSING_T, D), BF16)
        tabS[: len(uniqS)] = h16[uniqS]

        idxs = [np.full(nch[k] * P, pad_idx[k], np.int64) for k in range(NST)]
        dloc = [np.full((P, nmem_p[k]), PAD_DLOC, np.float32) for k in range(NST)]
        for j in range(NB):
            g = asg[j, c]
            for k in range(NST):
                sl0 = S[k, c, j]
                data = blk_slots[g][k]
                keys = data[0]
                n = len(keys)
                if n == 0:
                    continue
                sl = sl0 + np.arange(n)
                if k == 0:
                    idxs[k][sl] = np.array([smap[int(r)] for r in keys])
                else:
                    idxs[k][sl] = np.array([qmaps[k - 1][int(q)] for q in keys])
                pch = sl // P
                pid = np.array([pair_idx[k][(j, int(ch_))] for ch_ in pch])
                if k == 0:
                    dloc[k][sl % P, pid] = data[1] % P
                else:
                    dloc[k][sl % P, pid * 2] = data[1]
                    dloc[k][sl % P, pid * 2 + 1] = data[2]

        def idx_layout(v):
            w_ = v.astype(np.int16).reshape(-1, 16).T        # [16, S/16]
            return np.tile(w_, (8, 1)).copy()                # [128, S/16]

        hT = np.zeros((D, NPC), np.float32)
        rcp = np.zeros((1, NPC), np.float32)
        for j in range(NB):
            g = asg[j, c]
            lo2 = g * P
            hi2 = min(lo2 + P, N_NODES)
            if hi2 > lo2:
                hT[:, j * P : j * P + (hi2 - lo2)] = h[lo2:hi2].T
                rcp[0, j * P : j * P + (hi2 - lo2)] = recip[lo2:hi2]

        auxW = np.concatenate(
            [np.asarray(W1, np.float32), np.asarray(W2, np.float32)], axis=1
        ).astype(BF16)  # [128, 256]
        auxB = (np.asarray(b1, np.float32) + np.asarray(b2, np.float32))[:, None]

        in_maps.append(
            {
                "tabS": tabS,
                "tabPA": tabPs[0],
                "tabPB": tabPs[1],
                "idx0": idx_layout(idxs[0]),
                "idx1": idx_layout(idxs[1]),
                "idx2": idx_layout(idxs[2]),
                "dl0": dloc[0].astype(BF16),
                "dl1": dloc[1].astype(BF16),
                "dl2": dloc[2].astype(BF16),
                "hT": hT.astype(BF16),
                "rcp": rcp.astype(BF16),
                "ioI": iota_int,
                "auxW": auxW,
                "auxB": auxB.astype(np.float32),
            }
        )

    meta = dict(
        nch=nch,
        npairs=npairs,
        nmem=nmem,
        nmem_p=nmem_p,
        lo_r=lo_r,
        hi_r=hi_r,
        pair_idx=pair_idx,
        asg=asg,
    )
    return in_maps, meta


def _build(meta):
    import concourse.bacc as bacc
    import concourse.mybir as mybir
    import concourse.tile as tile

    nch = meta["nch"]
    nmem_p = meta["nmem_p"]
    lo_r, hi_r = meta["lo_r"], meta["hi_r"]
    pair_idx = meta["pair_idx"]
    f32 = mybir.dt.float32
    bf16 = mybir.dt.bfloat16

    nc = bacc.Bacc("TRN2", target_bir_lowering=False, debug=False, num_devices=NCORES)
    tabs_d = [
        nc.declare_dram_parameter("tabS", [NSING_T, D], bf16, isOutput=False),
        nc.declare_dram_parameter("tabPA", [NPAIR_T, 2 * D], bf16, isOutput=False),
        nc.declare_dram_parameter("tabPB", [NPAIR_T, 2 * D], bf16, isOutput=False),
    ]
    idx_d = [
        nc.declare_dram_parameter(f"idx{k}", [P, nch[k] * 8], mybir.dt.int16, isOutput=False)
        for k in range(NST)
    ]
    dl_d = [
        nc.declare_dram_parameter(f"dl{k}", [P, nmem_p[k]], bf16, isOutput=False)
        for k in range(NST)
    ]
    hT = nc.declare_dram_parameter("hT", [D, NPC], bf16, isOutput=False)
    rcp = nc.declare_dram_parameter("rcp", [1, NPC], bf16, isOutput=False)
    ioI = nc.declare_dram_parameter("ioI", [P, P * OHB], bf16, isOutput=False)
    auxW = nc.declare_dram_parameter("auxW", [P, 256], bf16, isOutput=False)
    auxB = nc.declare_dram_parameter("auxB", [P, 1], f32, isOutput=False)
    outT = nc.declare_dram_parameter("outT", [D, NPC], bf16, isOutput=True)

    def batch_schedule(n, bch):
        sched = []
        pos = 0
        while n - pos > 0:
            step = min(bch, n - pos)
            sched.append((pos, step))
            pos += step
        if sched and sched[-1][1] > 8:
            st, ln = sched.pop()
            sched.append((st, ln - 8))
            sched.append((st + ln - 8, 8))
        return sched

    bsched = [batch_schedule(nch[k], BCH[k]) for k in range(NST)]
    bmap = {}
    for k in range(NST):
        for bi, (st, ln) in enumerate(bsched[k]):
            for ch in range(st, st + ln):
                bmap[(k, ch)] = bi

    with tile.TileContext(nc) as tc, ExitStack() as ctx:
        consts = ctx.enter_context(tc.tile_pool(name="consts", bufs=1))
        g_pools = [
            ctx.enter_context(tc.tile_pool(name="g0", bufs=5)),
            ctx.enter_context(tc.tile_pool(name="g1", bufs=3)),
            ctx.enter_context(tc.tile_pool(name="g2", bufs=3)),
        ]
        oh_pools = [
            ctx.enter_context(tc.tile_pool(name="oh0", bufs=6)),
            ctx.enter_context(tc.tile_pool(name="oh1", bufs=3)),
            ctx.enter_context(tc.tile_pool(name="oh2", bufs=3)),
        ]
        hN_pool = ctx.enter_context(tc.tile_pool(name="hNp", bufs=4))
        out_pool = ctx.enter_context(tc.tile_pool(name="outp", bufs=8))
        psA = ctx.enter_context(tc.tile_pool(name="psA", bufs=5, space="PSUM"))
        psO = ctx.enter_context(tc.tile_pool(name="psO", bufs=3, space="PSUM"))

        HEAD = 2 * 32
        idx_t = []
        dl_t = []
        for k in range(NST):
            it = consts.tile([P, nch[k] * 8], mybir.dt.int16, name=f"idx{k}")
            h8 = min(HEAD * 8, nch[k] * 8)
            nc.sync.dma_start(it[:, :h8], idx_d[k][:, :h8])
            if nch[k] * 8 > h8:
                nc.sync.dma_start(it[:, h8:], idx_d[k][:, h8:])
            idx_t.append(it)
            dt_ = consts.tile([P, nmem_p[k]], bf16, name=f"dl{k}")
            hd = min(HEAD, nmem_p[k])
            nc.sync.dma_start(dt_[:, :hd], dl_d[k][:, :hd])
            if nmem_p[k] > hd:
                nc.sync.dma_start(dt_[:, hd:], dl_d[k][:, hd:])
            dl_t.append(dt_)
        auxW_t = consts.tile([P, 256], bf16)
        nc.sync.dma_start(auxW_t[:], auxW[:])
        auxB_t = consts.tile([P, 1], f32)
        nc.sync.dma_start(auxB_t[:], auxB[:])
        ioI_t = consts.tile([P, P * OHB], bf16)
        nc.sync.dma_start(ioI_t[:], ioI[:])
        rcp_t = consts.tile([1, NPC], bf16)
        nc.sync.dma_start(rcp_t[:], rcp[:])
        hT_all = consts.tile([P, NPC], bf16)
        nc.sync.dma_start(hT_all[:], hT[:])
        w1_s = auxW_t[:, 0:128]
        w2_s = auxW_t[:, 128:256]
        bias_s = auxB_t[:, 0:1]

        rb_t = consts.tile([P, NPC], bf16)
        nc.gpsimd.partition_broadcast(rb_t[:], rcp_t[:])

        g_tiles = [dict() for _ in range(NST)]
        oh_state = [(-1, None) for _ in range(NST)]

        def chunk_ap(k, j):
            """AP of gathered slot-chunk j of stream k ([P, EW[k]] view)."""
            b = bmap[(k, j)]
            st, ln = bsched[k][b]
            off = j - st
            ew = EW[k]
            tile_ = g_tiles[k].get(b)
            if tile_ is None:
                gt = g_pools[k].tile([P, BCH[k] * ew], bf16, name=f"g{k}")
                ni = ln * P
                nc.gpsimd.dma_gather(
                    gt[:, : ln * ew].rearrange("p (c e) -> p c e", e=ew),
                    tabs_d[k][:, :],
                    idx_t[k][:, st * 8 : st * 8 + (ni // 16)],
                    ni,
                    ni,
                    ew,
                    single_packet=False,
                )
                g_tiles[k][b] = gt
                tile_ = gt
            return tile_[:, off * ew : (off + 1) * ew]

        def oh_ap(k, mcol):
            """AP of member column mcol of stream k (stride-OHB view)."""
            b, ci = divmod(mcol, OHB)
            cur, tile_ = oh_state[k]
            if b != cur:
                ot = oh_pools[k].tile([P, P * OHB], bf16, name=f"oh{k}")
                in1 = (
                    dl_t[k][:, b * OHB : (b + 1) * OHB]
                    .rearrange("p (x c) -> p x c", x=1)
                    .broadcast_to([P, P, OHB])
                )
                nc.vector.tensor_tensor(
                    out=ot[:].rearrange("p (d c) -> p d c", c=OHB),
                    in0=ioI_t[:].rearrange("p (d c) -> p d c", c=OHB),
                    in1=in1,
                    op=mybir.AluOpType.is_equal,
                )
                oh_state[k] = (b, ot)
                tile_ = ot
            return tile_[:].rearrange("p (d c) -> p c d", c=OHB)[:, ci, :]

        for j in range(NB):
            agg = psA.tile([P, P], f32)
            plan = []
            for k in range(NST):
                for ch in range(int(lo_r[k][j]), int(hi_r[k][j])):
                    pr = pair_idx[k][(j, ch)]
                    for i in range(MC[k]):
                        plan.append((k, ch, pr * MC[k] + i, i))
            assert plan, f"empty plan at position {j}"
            for n_, (k, ch, mcol, half) in enumerate(plan):
                g_ap = chunk_ap(k, ch)
                nc.tensor.matmul(
                    agg[:],
                    lhsT=g_ap[:, half * D : (half + 1) * D],
                    rhs=oh_ap(k, mcol),
                    start=(n_ == 0),
                    stop=(n_ == len(plan) - 1),
                )

            hN_t = hN_pool.tile([P, P], bf16)
            nc.vector.tensor_tensor(
                out=hN_t[:],
                in0=agg[:],
                in1=rb_t[:, j * P : (j + 1) * P],
                op=mybir.AluOpType.mult,
            )

            po = psO.tile([P, P], f32)
            nc.tensor.matmul(
                po[:], lhsT=w1_s, rhs=hT_all[:, j * P : (j + 1) * P], start=True, stop=False
            )
            nc.tensor.matmul(po[:], lhsT=w2_s, rhs=hN_t[:], start=False, stop=True)

            if j % 2 == 0:
                out_t2 = out_pool.tile([P, 2 * P], bf16)
            out_sl = out_t2[:, 0:P] if j % 2 == 0 else out_t2[:, P : 2 * P]
            nc.scalar.activation(
                out_sl, po[:], mybir.ActivationFunctionType.Identity, bias=bias_s
            )
            if j % 2 == 1:
                nc.sync.dma_start(outT[:, (j - 1) * P : (j + 1) * P], out_t2[:])
            elif j == NB - 1:
                nc.sync.dma_start(outT[:, j * P : (j + 1) * P], out_t2[:, 0:P])

    nc.finalize()
    return nc


def kernel(h, src, dst, W1, b1, W2, b2):
    from concourse.bass_utils import run_bass_kernel_spmd

    in_maps, meta = _prep(h, src, dst, W1, b1, W2, b2)
    nc = _build(meta)
    res = run_bass_kernel_spmd(nc, in_maps, list(range(NCORES))).results
    return _assemble(res, meta)


def _assemble(res, meta):
    asg = meta["asg"]
    out = np.zeros((NCORES * NPC, D), np.float32)
    for j in range(NB):
        for c in range(NCORES):
            g = asg[j, c]
            out[g * P : (g + 1) * P] = (
                res[c]["outT"][:, j * P : (j + 1) * P].astype(np.float32).T
            )
    return out[:N_NODES].astype(np.float32)


def _sim(h, src, dst, W1, b1, W2, b2):
    """Numpy simulation of the exact device program (bookkeeping check)."""
    in_maps, meta = _prep(h, src, dst, W1, b1, W2, b2)
    lo_r, hi_r = meta["lo_r"], meta["hi_r"]
    pair_idx = meta["pair_idx"]
    outs = []
    for c in range(NCORES):
        m = in_maps[c]
        tabs = [m["tabS"], m["tabPA"], m["tabPB"]]
        idxs = []
        dloc = []
        for k in range(NST):
            w_ = m[f"idx{k}"][:16]
            idxs.append(w_.T.reshape(-1))
            dloc.append(m[f"dl{k}"].astype(np.float32))
        W1b = m["auxW"][:, 0:128].astype(np.float32)
        W2b = m["auxW"][:, 128:256].astype(np.float32)
        rcpv = m["rcp"].astype(np.float32)[0]
        outT = np.zeros((D, NPC), np.float32)
        for j in range(NB):
            agg = np.zeros((P, P), np.float32)
            for k in range(NST):
                for ch in range(int(lo_r[k][j]), int(hi_r[k][j])):
                    pr = pair_idx[k][(j, ch)]
                    gi = idxs[k][ch * P : (ch + 1) * P].astype(np.int64)
                    Gfull = tabs[k][gi].astype(np.float32)   # [128, EW[k]]
                    for i in range(MC[k]):
                        G = Gfull[:, i * D : (i + 1) * D]
                        dv = dloc[k][:, (pr * MC[k] + i)]
                        O = (dv[:, None] == np.arange(P)[None, :]).astype(np.float32)
                        agg += G.T @ O
            hNj = (agg * rcpv[None, j * P : (j + 1) * P]).astype(BF16).astype(np.float32)
            hTj = m["hT"][:, j * P : (j + 1) * P].astype(np.float32)
            po = W1b.T @ hTj + W2b.T @ hNj
            outT[:, j * P : (j + 1) * P] = (po + m["auxB"]).astype(BF16)
        outs.append({"outT": outT.astype(BF16)})
    return _assemble(outs, meta)


if __name__ == "__main__":
    rng = np.random.default_rng(0)
    h = rng.standard_normal((N_NODES, D), dtype=np.float32)
    src = rng.integers(0, N_NODES, N_EDGES)
    dst = rng.integers(0, N_NODES, N_EDGES)
    W1 = rng.standard_normal((D, D), dtype=np.float32) * 0.1
    b1 = rng.standard_normal(D, dtype=np.float32) * 0.1
    W2 = rng.standard_normal((D, D), dtype=np.float32) * 0.1
    b2 = rng.standard_normal(D, dtype=np.float32) * 0.1

    msgs_sum = np.zeros((N_NODES, D), np.float32)
    np.add.at(msgs_sum, dst, h[src])
    deg = np.bincount(dst, minlength=N_NODES).astype(np.float32)
    hN = msgs_sum / np.maximum(deg, 1.0)[:, None]
    ref = h @ W1 + b1 + hN @ W2 + b2

    got = _sim(h, src, dst, W1, b1, W2, b2)
    err = np.linalg.norm(got - ref) / np.linalg.norm(ref)
    print("sim rel err (norm):", err)
    print("sim max abs err:", np.abs(got - ref).max())
    in_maps, meta = _prep(h, src, dst, W1, b1, W2, b2)
    print("nch:", meta["nch"], "npairs:", meta["npairs"])
    tot_slots = sum(meta["nch"][k] * 128 for k in range(NST))
    print("slot-descs/core:", tot_slots)
